# revision 31
# baseline (speedup 1.0000x reference)
# DSTP-RNN Trainium2 kernel: 8-core pure data parallel (batch 512 -> 64/core).
#
# Layout summary:
#  - "Score" tensors are b-major: partitions = (g, b) with g a channel-group
#    split, b = 64 local batch rows; free dims = (ch, tau).
#  - LSTM runs H-major: gates land in PSUM [H=128, b=64] via per-quarter
#    matmuls (stationary = weight slices), so h/c are produced directly in
#    the [H, b] layout the next step's matmuls consume - no transposes on
#    the recurrent chain.  States are doubled (hS=2h, cS=2c) and sigmoids
#    are computed as tanh(x/2) with pre-scaled weights.
#  - All matmul operands are bf16 (1 cycle/row); elementwise stays f32.
#  - Gate accumulation is split: bias+h parts issue right after the e-matmul
#    (overlapping the attention score), x parts join at the end.
#  - Off-critical-path copies (finB, WxF, mid2T staging) go to ACT/Pool.
import numpy as np
import ml_dtypes

import concourse.bacc as bacc
import concourse.mybir as mybir
import concourse.tile as tile
from concourse.bass_utils import run_bass_kernel_spmd

F32 = mybir.dt.float32
BF16 = mybir.dt.bfloat16
AX = mybir.AxisListType
OP = mybir.AluOpType
AF = mybir.ActivationFunctionType

N_CORES = 8
B = 64      # batch per core
T = 64      # encoder length
H = 128
TD = 24     # decoder steps (T_DEC + 6)
NF = 17     # driving series count
C2 = 129    # stage-2 channels (H + label)
COLS = np.array(list(range(14)) + list(range(15, 18)))
PAD_NEG = -20.0   # pad channel fill (tanh -> -1; excluded from softmax sums)


def _perm_cols(w):
    # keep torch gate order (i,f,g,o): i,f,g contiguous so the chain-critical
    # tanh (i,f,g) can issue before the o quarter's matmuls finish
    return w


def _bf(x):
    return np.ascontiguousarray(np.asarray(x).astype(ml_dtypes.bfloat16))


def _f32(x):
    return np.ascontiguousarray(np.asarray(x).astype(np.float32))


def prep_weights(inp):
    w = {}
    w["Wi1R"] = _bf(np.concatenate([inp["Wi_w"].T, inp["Wi_b"][None, :]], 0))
    w["Wi2R"] = _bf(np.concatenate([inp["Wi2_w"].T * 0.5, inp["Wi2_b"][None, :]], 0))
    w["We1R"] = _bf(inp["We_w"].T * 0.5)
    w["We2R"] = _bf(inp["We2_w"].T * 0.5)
    w["WhR"] = _bf(inp["Wh_w"].T * 0.5)
    w["WxR"] = _bf(inp["Wx_w"].T * 0.5)
    w["Wxb"] = _bf(inp["Wx_b"][None, :])

    # ISO: sigmoid gates computed as tanh(x/2) -> pre-scale i,f,o cols by 0.5.
    # States are stored doubled (hS=2h, cS=2c), so weight blocks consuming
    # h/c/mid/din get an extra 0.5.
    ISO = np.concatenate([0.5 * np.ones(256), np.ones(128),
                          0.5 * np.ones(128)]).astype(np.float32)
    g1x = _perm_cols(inp["Wih1"].T) * ISO
    b1 = _perm_cols((inp["bih1"] + inp["bhh1"])[None, :]) * ISO
    w["G1XA"] = _bf(np.concatenate([g1x[0:9], b1], 0))   # +bias row (ones in x)
    w["G1XB"] = _bf(g1x[9:17])
    w["G1H"] = _bf(_perm_cols(inp["Whh1"].T) * ISO * 0.5)

    g2x = _perm_cols(inp["Wih2"].T) * ISO * 0.5
    b2 = _perm_cols((inp["bih2"] + inp["bhh2"])[None, :]) * ISO
    # stage-2 channel groups are chosen so x rows align with hT1 partitions:
    # group 0 = [h0..h63, label] (65 ch), group 1 = [h64..h127] (64 ch)
    w["G2XA"] = _bf(np.concatenate([g2x[0:64], g2x[128:129], b2], 0))  # +bias row
    w["G2XB"] = _bf(g2x[64:128])
    w["G2H"] = _bf(_perm_cols(inp["Whh2"].T) * ISO * 0.5)

    w["GdX"] = _bf(_perm_cols(inp["Wihd"].T) * ISO * 0.5)
    w["GdH"] = _bf(_perm_cols(inp["Whhd"].T) * ISO * 0.5)
    w["bdrow"] = _bf(_perm_cols((inp["bihd"] + inp["bhhd"])[None, :]) * ISO)

    w["vdup1"] = _bf(np.broadcast_to(inp["Vd_w"][0][None, :], (128, T)))
    w["vdup2"] = _bf(np.broadcast_to(inp["Vd2_w"][0][None, :], (128, T)))
    w["vdup3"] = _bf(np.broadcast_to(inp["V_w"][0][None, :], (128, H)))
    w["regw"] = _bf(inp["reg_w"][0][:, None] * 0.5)

    eye = np.eye(64, dtype=np.float32)
    w["I64dup"] = _f32(np.concatenate([eye, eye], 0))
    w["I64bf"] = _bf(np.concatenate([eye, eye], 0))
    w["I128bf"] = _bf(np.eye(128, dtype=np.float32))
    foldDup = (np.arange(128)[:, None] % 64 == np.arange(128)[None, :] % 64)
    w["foldDup"] = _f32(foldDup.astype(np.float32))
    w["onesrow"] = _bf(np.ones((1, 64), np.float32))
    return w


def prep_core_inputs(inp, core):
    b0, b1 = core * B, (core + 1) * B
    x = np.asarray(inp["input_p_q"])[b0:b1, :T, :][:, :, COLS]   # [64,64,17]
    lab = np.asarray(inp["label_p"])[b0:b1, :T]                  # [64,64]
    d = {}
    inpT = np.ones((65, NF * B), np.float32)
    inpT[:64] = x.transpose(1, 2, 0).reshape(64, NF * B)         # [t, (c,b)]
    d["inpT"] = _bf(inpT)
    ct = x.transpose(2, 1, 0).reshape(NF, T * B)                 # [c, (t,b)]
    d["inpCTA"] = _bf(ct[0:9])
    d["inpCTB"] = _bf(ct[9:17])
    d["labelT"] = _f32(lab.T * 2.0)                                    # [t, b]
    return d


DRAM_SPECS = {
    "inpT": ([65, NF * B], BF16), "inpCTA": ([9, T * B], BF16),
    "inpCTB": ([8, T * B], BF16), "labelT": ([T, B], F32),
    "Wi1R": ([65, 64], BF16), "Wi2R": ([65, 64], BF16),
    "We1R": ([256, 64], BF16), "We2R": ([256, 64], BF16),
    "WhR": ([256, 128], BF16), "WxR": ([128, 128], BF16), "Wxb": ([1, 128], BF16),
    "G1XA": ([10, 512], BF16), "G1XB": ([8, 512], BF16), "G1H": ([128, 512], BF16),
    "G2XA": ([66, 512], BF16), "G2XB": ([64, 512], BF16), "G2H": ([128, 512], BF16),
    "GdX": ([128, 512], BF16), "GdH": ([128, 512], BF16), "bdrow": ([1, 512], BF16),
    "vdup1": ([128, T], BF16), "vdup2": ([128, T], BF16), "vdup3": ([128, H], BF16),
    "regw": ([128, 1], BF16), "I64dup": ([128, 64], F32), "I64bf": ([128, 64], BF16),
    "I128bf": ([128, 128], BF16), "foldDup": ([128, 128], F32),
    "onesrow": ([1, 64], BF16),
}


def build_nc(num_devices=N_CORES, skip_score=False, skip_tail=False, only_stages=(1, 2, 3), split=(0.28, 0.64), split3=(0.25, 0.62)):
    nc = bacc.Bacc("TRN2", target_bir_lowering=False, debug=False,
                   num_devices=num_devices)
    dr = {}
    for name, (shape, dt) in DRAM_SPECS.items():
        dr[name] = nc.dram_tensor(name, shape, dt, kind="ExternalInput").ap()
    out_d = nc.dram_tensor("out", [B, 18], F32, kind="ExternalOutput").ap()

    with tile.TileContext(nc) as tc:
        # ---------- persistent SBUF ----------
        wpool = tc.alloc_tile_pool(name="wpool", bufs=1)
        sb = {}
        for name, (shape, dt) in DRAM_SPECS.items():
            if shape[0] > 128:
                assert shape[0] == 256
                for half, suf in ((0, "a"), (1, "b")):
                    key = name + suf
                    sb[key] = wpool.tile([128, shape[1]], dt, name=f"sb_{key}")
                    nc.sync.dma_start(sb[key][:], dr[name][128 * half:128 * (half + 1), :])
            else:
                sb[name] = wpool.tile(shape, dt, name=f"sb_{name}")
                nc.sync.dma_start(sb[name][:], dr[name][:])

        G2XBsh = wpool.tile([128, 512], BF16, name="G2XBsh")
        nc.sync.dma_start(G2XBsh[64:128, :], dr["G2XB"][:])
        X1 = wpool.tile([128, 9, T], BF16, name="X1")
        X2 = wpool.tile([128, 65, T], BF16, name="X2")
        WxF3 = wpool.tile([128, 32, H], BF16, name="WxF3")
        finB = wpool.tile([128, H, 32], BF16, name="finB")
        finT = wpool.tile([128, T, B], BF16, name="finT")    # stage-2 h, H-major
        hT1 = wpool.tile([128, T, B], BF16, name="hT1")      # stage-1 h, H-major
        labB = wpool.tile([65, T, B], BF16, name="labB")     # label at partition 64
        mid2T = wpool.tile([65, B, C2], BF16, name="mid2T")
        xA1pp = [wpool.tile([10, 64], BF16, name=f"xA1_{k}") for k in range(2)]
        xA2pp = [wpool.tile([66, 64], BF16, name=f"xA2_{k}") for k in range(2)]
        zeros128 = wpool.tile([128, 128], F32, name="zeros128")
        zerobf = wpool.tile([128, 64], BF16, name="zerobf")
        ones1 = wpool.tile([1, 64], BF16, name="ones1")
        outsb = wpool.tile([B, 18], F32, name="outsb")

        for k in range(2):
            nc.gpsimd.dma_start(xA1pp[k][9:10, :], dr["onesrow"][:])
            nc.gpsimd.dma_start(xA2pp[k][65:66, :], dr["onesrow"][:])
        nc.vector.memset(zerobf[:], 0.0)
        nc.vector.memset(zeros128[:], 0.0)
        nc.vector.memset(ones1[:], 1.0)
        nc.vector.memset(mid2T[64:65, :, :], 1.0)
        nc.vector.memset(X2[64:128, 64, :], PAD_NEG)
        nc.vector.memset(X1[64:128, 8, :], PAD_NEG)
        # label -> mid2T[t, b, 128] and labB[64, t, b]
        nc.gpsimd.dma_start(mid2T[0:64, :, 128:129], dr["labelT"][:])
        nc.gpsimd.dma_start(labB[64:65, :, :], dr["labelT"][:])

        if only_stages != (1, 2, 3):
            # profiling variants: init tiles a skipped stage would have written
            nc.vector.memset(finT[:], 0.1)
            nc.vector.memset(finB[:], 0.1)
            nc.vector.memset(hT1[:], 0.1)
            nc.vector.memset(mid2T[:], 0.1)
            nc.vector.memset(X2[:], 0.1)
            nc.vector.memset(X1[:], 0.1)
            nc.vector.memset(WxF3[:], 0.1)
            nc.vector.memset(outsb[:], 0.0)

        # ---------- X1 build ----------
        with tc.tile_pool(name="xb1", space="PSUM", bufs=1) as xb:
            x1ps = xb.tile([128, 9, T], F32, name="x1ps")
            for c in range(NF):
                g, ch = (0, c) if c < 9 else (1, c - 9)
                rows = slice(g * 64, g * 64 + 64)
                nc.tensor.matmul(x1ps[rows, ch, :],
                                 sb["inpT"][:, c * B:(c + 1) * B],
                                 sb["Wi1R"][:], start=True, stop=True)
            nc.vector.tensor_copy(X1[0:64, :, :], x1ps[0:64, :, :])
            nc.scalar.copy(X1[64:128, 0:8, :], x1ps[64:128, 0:8, :])

        # ================= helpers =================
        def lstm_hmajor(gps, cH_old, hdst, sp, pfx=""):
            """H-major doubled-state LSTM.  gps psum [128, 256] = [i|f|o|g]
            quarters, each [H=128, b=64].  Writes hS (bf16) into hdst and
            returns (cH_new f32, cTbf bf16)."""
            ta = sp.tile([128, 4, 64], F32, name=pfx + "ta", tag=pfx + "ta", bufs=2)
            # i,f,g first (feeds the c update); o separately (only needed by h)
            nc.scalar.activation(ta[:, 0:3, :], gps[:, 0:3, 0:64], AF.Tanh)
            nc.scalar.activation(ta[:, 3:4, :], gps[:, 3:4, 0:64], AF.Tanh)
            u = sp.tile([128, 64], F32, name=pfx + "u", tag=pfx + "u", bufs=2)
            v2 = sp.tile([128, 64], F32, name=pfx + "v2", tag=pfx + "v2", bufs=2)
            # u = (tanh(i/2)+1)*tanh(g) = 2*sig(i)*tanh(g)
            nc.vector.scalar_tensor_tensor(u[:], ta[:, 0, :], 1.0,
                                           ta[:, 2, :], op0=OP.add, op1=OP.mult)
            # v = (tanh(f/2)+1)*cS = 4*sig(f)*c
            nc.vector.scalar_tensor_tensor(v2[:], ta[:, 1, :], 1.0,
                                           cH_old[:], op0=OP.add, op1=OP.mult)
            cH = sp.tile([128, 64], F32, name=pfx + "cH", tag=pfx + "cH", bufs=2)
            # cS_new = v/2 + u = 2*c_new
            nc.vector.scalar_tensor_tensor(cH[:], v2[:], 0.5,
                                           u[:], op0=OP.mult, op1=OP.add)
            cTbf = sp.tile([128, 64], BF16, name=pfx + "cTb", tag=pfx + "cTb", bufs=2)
            nc.vector.tensor_copy(cTbf[:], cH[:])
            tcel = sp.tile([128, 64], F32, name=pfx + "tc", tag=pfx + "tc", bufs=2)
            nc.scalar.activation(tcel[:], cH[:], AF.Tanh, scale=0.5)
            # hS_new = (tanh(o/2)+1)*tanh(c) = 2*h_new
            nc.vector.scalar_tensor_tensor(hdst, ta[:, 3, :], 1.0,
                                           tcel[:], op0=OP.add, op1=OP.mult)
            return cH, cTbf

        def softmax_nomax(score, pool, ppool, nch, ptag="tps", pbufs=3,
                          want_a=True, exdt=F32):
            # score pad slots (if any) must already be ~-30 so exp ~ 0;
            # accum_out fuses the per-partition sum into the exp pass.
            ex = pool.tile([128, nch], exdt, name="ex", tag="sm_ex", bufs=2)
            zs = pool.tile([128, 1], F32, name="zs", tag="sm_zs", bufs=2)
            nc.scalar.activation(ex[:], score[:], AF.Exp, accum_out=zs[:])
            zps = ppool.tile([128, 1], F32, name="zps", tag=ptag, bufs=pbufs)
            nc.tensor.matmul(zps[:], sb["foldDup"][:], zs[:], start=True, stop=True)
            zr = pool.tile([128, 1], F32, name="zr", tag="sm_zr", bufs=2)
            nc.vector.reciprocal(zr[:], zps[:])
            if not want_a:
                return ex, zr
            a = pool.tile([128, nch], BF16, name="a", tag="sm_a", bufs=2)
            nc.vector.tensor_scalar_mul(a[:], ex[:], zr[:])
            return a

        def tree_to(dst, src, pool, tag, nch, ntau, single_cut=0, eng=None):
            """sum src [128, nch, ntau] over tau into dst [128, nch] slice."""
            if eng is None:
                eng = nc.vector
            if eng is not nc.vector:
                single_cut = 0   # Pool tensor_reduce can't do innermost-axis
            cur, n, lvl = src, ntau, 0
            while n > max(2, single_cut):
                n //= 2
                nxt = pool.tile([128, nch, n], BF16, name=f"{tag}_{lvl}",
                                tag=f"{tag}_{lvl}", bufs=1)
                eng.tensor_add(nxt[:], cur[:, :, 0:n], cur[:, :, n:2 * n])
                cur, lvl = nxt, lvl + 1
            if n > 2:
                with nc.allow_low_precision(reason="tiny bf16 tau-reduce"):
                    nc.vector.tensor_reduce(dst, cur[:], AX.X, OP.add)
            else:
                eng.tensor_add(dst.unsqueeze(-1), cur[:, :, 0:1], cur[:, :, 1:2])

        def score_chunked(Xs, esb, vdup, nch, ntau, sp, tag, pad_neg=False,
                          nchunks=2):
            """returns score [128, nch] bf16; chunks over ch for engine overlap."""
            score = sp.tile([128, nch], BF16, name="score", tag=f"{tag}_score",
                            bufs=2)
            if nchunks == 1:
                bounds = ((0, nch),)
            elif isinstance(nchunks, tuple):
                cuts = [0] + [max(1, min(nch - 1, int(round(nch * f)))) for f in nchunks] + [nch]
                bounds = tuple((cuts[i], cuts[i + 1]) for i in range(len(cuts) - 1))
            elif isinstance(nchunks, float):
                cut = max(1, min(nch - 1, int(round(nch * nchunks))))
                bounds = ((0, cut), (cut, nch))
            elif nchunks == 2:
                half = (nch + 1) // 2
                bounds = ((0, half), (half, nch))
            else:
                q = max(1, nch // nchunks)
                cuts = list(range(0, nch, q))
                bounds = tuple((lo, min(lo + q, nch)) for lo in cuts)
            for ci, (lo, hi) in enumerate(bounds):
                w = hi - lo
                scA = sp.tile([128, w, ntau], BF16, name="scA",
                              tag=f"{tag}_scA{lo}", bufs=1)
                nc.vector.tensor_add(scA[:], Xs[:, lo:hi, :],
                                     esb[:].unsqueeze(1).broadcast_to([128, w, ntau]))
                scT = sp.tile([128, w, ntau], BF16, name="scT",
                              tag=f"{tag}_scT{lo}", bufs=1)
                nc.scalar.activation(scT[:], scA[:], AF.Tanh)
                scM = sp.tile([128, w, ntau], BF16, name="scM",
                              tag=f"{tag}_scM{lo}", bufs=1)
                # the first chunk's result isn't needed until the exp, so its
                # mul+tree can run on the otherwise-idle Pool engine
                eng = nc.gpsimd if (ci == 0 and len(bounds) > 1) else nc.vector
                eng.tensor_mul(scM[:], scT[:],
                               vdup[:].unsqueeze(1).broadcast_to([128, w, ntau]))
                tree_to(score[:, lo:hi], scM, sp, f"{tag}_tr{lo}", w, ntau,
                        single_cut=8, eng=eng)
            if pad_neg:
                # kill the (g=1, ch=nch-1) pad slot before exp
                nc.vector.memset(score[64:128, nch - 1:nch], -30.0)
            return score

        # ================= encoder step =================
        def enc_step(t, stage, sp, pp, st):
            if stage == 1:
                Xs, vdup, WeRa, WeRb = X1, sb["vdup1"], sb["We1Ra"], sb["We1Rb"]
                nch = 9
                GH, GXA, GXB = sb["G1H"], sb["G1XA"], sb["G1XB"]
            else:
                Xs, vdup, WeRa, WeRb = X2, sb["vdup2"], sb["We2Ra"], sb["We2Rb"]
                nch = 65
                GH, GXA, GXB = sb["G2H"], sb["G2XA"], G2XBsh
            hT_old, cT_old, cH_old = st["hT"], st["cT"], st["cH"]
            tpsb = 2

            # e = [h;c] @ We  (b-major psum); c-part first (it's ready earlier)
            eps = pp.tile([128, T], F32, name="eps", tag="eps", bufs=1)
            for gb in (0, 64):
                o = eps[gb:gb + 64, :]
                nc.tensor.matmul(o, cT_old[:], WeRb[:], start=True, stop=False)
                nc.tensor.matmul(o, hT_old[:], WeRa[:], start=False, stop=True)
            esb = sp.tile([128, T], BF16, name="esb", tag="esb", bufs=2)
            nc.vector.tensor_copy(esb[:], eps[:])
            for fn in st.pop("defer", []):
                fn()

            # one bank per gate quarter: a start=True only zeroes its own bank
            gps = pp.tile([128, 4, 512], F32, name="gps", tag="gps", bufs=1)
            for q in range(4):
                nc.tensor.matmul(gps[:, q, 0:64], GH[:, 128 * q:128 * (q + 1)],
                                 hT_old[:], start=True, stop=False)

            if skip_score:
                score = sp.tile([128, nch], BF16, name="score", tag="e_score", bufs=2)
                nc.vector.memset(score[:], 0.1)
            else:
                score = score_chunked(Xs, esb, vdup, nch, T, sp, "e", pad_neg=True,
                                      nchunks=1 if stage == 1 else split)
            a = softmax_nomax(score, sp, pp, nch, pbufs=tpsb, exdt=BF16)

            if stage == 1:
                aTA = pp.tile([9, 64], BF16, name="aTA", tag="tps", bufs=tpsb)
                nc.tensor.transpose(aTA[:], a[0:64, 0:9], sb["I64bf"][0:64, :])
                aTB = pp.tile([8, 64], BF16, name="aTB", tag="tps", bufs=tpsb)
                nc.tensor.transpose(aTB[:], a[64:128, 0:8], sb["I64bf"][64:128, :])
                xB = sp.tile([8, 64], BF16, name="x1B", tag="xB", bufs=2)
                nc.vector.tensor_mul(xB[:], sb["inpCTB"][:, t * B:(t + 1) * B], aTB[:])
                xA = xA1pp[t % 2]
                nc.vector.tensor_mul(xA[0:9, :],
                                     sb["inpCTA"][:, t * B:(t + 1) * B], aTA[:])
            else:
                # group 0 = [h0..h63, label] at partitions 0..64,
                # group 1 = [h64..h127] at partitions 64..127 (psum base 64)
                aTA = pp.tile([65, 64], BF16, name="aTA", tag="tps", bufs=tpsb)
                nc.tensor.transpose(aTA[:], a[0:64, 0:65], sb["I64bf"][0:64, :])
                aTB = pp.tile([64, 64], BF16, name="aTB", tag="tps", bufs=tpsb)
                nc.tensor.transpose(aTB[:], a[64:128, 0:64], sb["I64bf"][64:128, :])
                xB = sp.tile([128, 64], BF16, name="x2B", tag="xB", bufs=2)
                nc.vector.tensor_mul(xB[64:128, :], hT1[64:128, t, :], aTB[0:64, :])
                xA = xA2pp[t % 2]
                nc.vector.tensor_mul(xA[0:64, :], hT1[0:64, t, :], aTA[0:64, :])
                nc.vector.tensor_mul(xA[64:65, :], labB[64:65, t, :], aTA[64:65, :])

            # late gate parts join the open per-bank groups; GXB first (xB is
            # the first x-mul to finish, so PE overlaps the remaining muls)
            for q in range(4):
                o = gps[:, q, 0:64]
                if stage == 1:
                    nc.tensor.matmul(o, GXB[:, 128 * q:128 * (q + 1)], xB[:],
                                     start=False, stop=False, skip_group_check=True)
                else:
                    nc.tensor.matmul(o, GXB[64:128, 128 * q:128 * (q + 1)],
                                     xB[64:128, :], start=False, stop=False,
                                     skip_group_check=True)
            for q in range(4):
                nc.tensor.matmul(gps[:, q, 0:64], GXA[:, 128 * q:128 * (q + 1)],
                                 xA[:], start=False, stop=True,
                                 skip_group_check=True)

            hdst = hT1[:, t, :] if stage == 1 else finT[:, t, :]
            cH, cTbf = lstm_hmajor(gps, cH_old, hdst, sp)
            st["hT"], st["cT"], st["cH"] = hdst, cTbf, cH

            # ---- off-critical-path per-step outputs; the ACT copies are
            # deferred past the next step's esb so they don't delay it ----
            if stage == 1:
                # mid2T[t] = h^T (b-major) for the X2 build
                def _defer(t=t, hdst=hdst):
                    hBps = pp.tile([64, 128], BF16, name="hBps", tag="hBps", bufs=1)
                    nc.tensor.transpose(hBps[:], hdst, sb["I128bf"][:])
                    hbf = sp.tile([64, 128], BF16, name="hbf", tag="hbf", bufs=2)
                    nc.scalar.copy(hbf[:], hBps[:])
                    if t % 2 == 0:
                        nc.sync.dma_start(mid2T[t:t + 1, :, 0:128], hbf[:])
                    else:
                        nc.gpsimd.dma_start(mid2T[t:t + 1, :, 0:128], hbf[:])
                st["defer"] = [_defer]
            else:
                g, sl = divmod(t, 32)
                rows = slice(g * 64, g * 64 + 64)
                # finB[(g,b), H, sl] = h^T  (transpose lands at base 0; the
                # copy shifts it to the group's partition block)
                def _defer(rows=rows, sl=sl, hdst=hdst):
                    fps = pp.tile([64, 128], BF16, name="fps", tag="fw", bufs=1)
                    nc.tensor.transpose(fps[:], hdst, sb["I128bf"][:])
                    wxps = pp.tile([128, 128], F32, name="wxps", tag="fw", bufs=1)
                    nc.tensor.matmul(wxps[rows, :], hdst, sb["WxR"][:],
                                     start=True, stop=True)
                    nc.scalar.copy(finB[rows, :, sl].unsqueeze(-1),
                                   fps[:].unsqueeze(-1))
                    nc.scalar.copy(WxF3[rows, sl, :], wxps[rows, :])
                st["defer"] = [_defer]

        # ---------- stage 1 ----------
        with tc.tile_pool(name="s1sp", bufs=2) as sp, \
             tc.tile_pool(name="s1pp", space="PSUM", bufs=2) as pp:
            st = {"hT": zerobf, "cT": zerobf, "cH": zeros128[:, 0:64]}
            for t in range(T if 1 in only_stages else 0):
                enc_step(t, 1, sp, pp, st)
            for fn in st.pop("defer", []):
                fn()

        # ---------- X2 build ----------
        with tc.tile_pool(name="xb2", space="PSUM", bufs=2) as xb2:
            for r in range(4):
                x2ps = xb2.tile([128, 16, T], F32, name="x2ps", tag="x2ps", bufs=2)
                for k in range(16):
                    ch = r * 16 + k
                    nc.tensor.matmul(x2ps[0:64, k, :], mid2T[:, :, ch],
                                     sb["Wi2R"][:], start=True, stop=True)
                    nc.tensor.matmul(x2ps[64:128, k, :], mid2T[:, :, 64 + ch],
                                     sb["Wi2R"][:], start=True, stop=True)
                nc.vector.tensor_copy(X2[:, r * 16:(r + 1) * 16, :], x2ps[:])
            x2ps2 = xb2.tile([64, T], F32, name="x2ps2", tag="x2ps2", bufs=1)
            nc.tensor.matmul(x2ps2[:], mid2T[:, :, 128], sb["Wi2R"][:],
                             start=True, stop=True)
            nc.vector.tensor_copy(X2[0:64, 64, :], x2ps2[:])

        # ---------- stage 2 ----------
        with tc.tile_pool(name="s2sp", bufs=2) as sp, \
             tc.tile_pool(name="s2pp", space="PSUM", bufs=2) as pp:
            st = {"hT": zerobf, "cT": zerobf, "cH": zeros128[:, 0:64]}
            for t in range(T if 2 in only_stages else 0):
                enc_step(t, 2, sp, pp, st)
            for fn in st.pop("defer", []):
                fn()

        # ---------- stage 3 ----------
        with tc.tile_pool(name="s3sp", bufs=2) as sp, \
             tc.tile_pool(name="s3pp", space="PSUM", bufs=2) as pp:
            outps = pp.tile([64, 18], F32, name="outps", bufs=1) if 3 in only_stages else None
            st = {"hT": zerobf, "cT": zerobf, "cH": zeros128[:, 0:64]}
            for t in range(TD if 3 in only_stages else 0):
                hT_old, cT_old, cH_old = st["hT"], st["cT"], st["cH"]
                eps = pp.tile([128, H], F32, name="e3ps", tag="eps3", bufs=1)
                for gb in (0, 64):
                    o = eps[gb:gb + 64, :]
                    nc.tensor.matmul(o, ones1[:], sb["Wxb"][:], start=True, stop=False)
                    nc.tensor.matmul(o, cT_old[:], sb["WhRb"][:],
                                     start=False, stop=False)
                    nc.tensor.matmul(o, hT_old[:], sb["WhRa"][:],
                                     start=False, stop=True)
                esb = sp.tile([128, H], BF16, name="e3sb", tag="esb3", bufs=2)
                nc.vector.tensor_copy(esb[:], eps[:])

                gps = pp.tile([128, 4, 512], F32, name="g3ps", tag="g3ps", bufs=1)
                for q in range(4):
                    o = gps[:, q, 0:64]
                    nc.tensor.matmul(o, sb["bdrow"][:, 128 * q:128 * (q + 1)],
                                     ones1[:], start=True, stop=False)
                    nc.tensor.matmul(o, sb["GdH"][:, 128 * q:128 * (q + 1)],
                                     hT_old[:], start=False, stop=False)

                if skip_score:
                    score = sp.tile([128, 32], BF16, name="score", tag="d_score", bufs=2)
                    nc.vector.memset(score[:], 0.1)
                else:
                    score = score_chunked(WxF3, esb, sb["vdup3"], 32, H, sp, "d",
                                          nchunks=split3 if split3 is not None else split)
                ex, zr = softmax_nomax(score, sp, pp, 32, ptag="tps3", pbufs=2,
                                       want_a=False, exdt=BF16)

                # context from unnormalized ex (starts right after exp; the
                # fold/recip run in parallel), then scale uu by 1/Z
                # (per-partition) before the fold+transpose matmul.
                ym = sp.tile([128, H, 32], BF16, name="ym", tag="ym", bufs=1)
                nc.vector.tensor_mul(ym[:], finB[:],
                                     ex[:].unsqueeze(1).broadcast_to([128, H, 32]))
                uu0 = sp.tile([128, H], BF16, name="uu0", tag="uu0", bufs=2)
                tree_to(uu0[:], ym, sp, "ctr", H, 32, single_cut=0)
                uu = sp.tile([128, H], BF16, name="uu", tag="uu", bufs=2)
                nc.vector.tensor_scalar_mul(uu[:], uu0[:], zr[:])
                dinps = pp.tile([128, 64], F32, name="dinps", tag="tps3", bufs=2)
                nc.tensor.matmul(dinps[:], uu[:], sb["I64bf"][:],
                                 start=True, stop=True)
                dinT = sp.tile([128, 64], BF16, name="dinT", tag="dinT", bufs=2)
                nc.vector.tensor_copy(dinT[:], dinps[:])

                # late gate part: din joins the open per-bank groups
                for q in range(4):
                    nc.tensor.matmul(gps[:, q, 0:64],
                                     sb["GdX"][:, 128 * q:128 * (q + 1)],
                                     dinT[:], start=False, stop=True,
                                     skip_group_check=True)

                h3 = sp.tile([128, 64], BF16, name="h3", tag="h3", bufs=2)
                cH, cTbf = lstm_hmajor(gps, cH_old, h3[:], sp, pfx="3")
                st["hT"], st["cT"], st["cH"] = h3, cTbf, cH

                if t >= TD - 18:
                    j = t - (TD - 18)
                    nc.tensor.matmul(outps[:, j:j + 1], h3[:], sb["regw"][:],
                                     start=True, stop=True)

            if 3 in only_stages:
                nc.vector.tensor_copy(outsb[:], outps[:])
            nc.sync.dma_start(out_d[:], outsb[:])

        wpool.release()

    nc.compile()
    return nc


_NC_CACHE = {}


def kernel(**inputs):
    if "nc" not in _NC_CACHE:
        _NC_CACHE["nc"] = build_nc()
    nc = _NC_CACHE["nc"]
    w = prep_weights({k: np.asarray(v) for k, v in inputs.items()})
    in_maps = []
    for core in range(N_CORES):
        m = dict(w)
        m.update(prep_core_inputs(inputs, core))
        in_maps.append(m)
    res = run_bass_kernel_spmd(nc, in_maps, list(range(N_CORES)))
    out = np.concatenate([res.results[c]["out"] for c in range(N_CORES)], axis=0)
    out = out + np.asarray(inputs["reg_b"])[0]
    return out.astype(np.float32)


# revision 32
# speedup vs baseline: 1.0033x; 1.0033x over previous
# DSTP-RNN Trainium2 kernel: 8-core pure data parallel (batch 512 -> 64/core).
#
# Layout summary:
#  - "Score" tensors are b-major: partitions = (g, b) with g a channel-group
#    split, b = 64 local batch rows; free dims = (ch, tau).
#  - LSTM runs H-major: gates land in PSUM [H=128, b=64] via per-quarter
#    matmuls (stationary = weight slices), so h/c are produced directly in
#    the [H, b] layout the next step's matmuls consume - no transposes on
#    the recurrent chain.  States are doubled (hS=2h, cS=2c) and sigmoids
#    are computed as tanh(x/2) with pre-scaled weights.
#  - All matmul operands are bf16 (1 cycle/row); elementwise stays f32.
#  - Gate accumulation is split: bias+h parts issue right after the e-matmul
#    (overlapping the attention score), x parts join at the end.
#  - Off-critical-path copies (finB, WxF, mid2T staging) go to ACT/Pool.
import numpy as np
import ml_dtypes

import concourse.bacc as bacc
import concourse.mybir as mybir
import concourse.tile as tile
from concourse.bass_utils import run_bass_kernel_spmd

F32 = mybir.dt.float32
BF16 = mybir.dt.bfloat16
AX = mybir.AxisListType
OP = mybir.AluOpType
AF = mybir.ActivationFunctionType

N_CORES = 8
B = 64      # batch per core
T = 64      # encoder length
H = 128
TD = 24     # decoder steps (T_DEC + 6)
NF = 17     # driving series count
C2 = 129    # stage-2 channels (H + label)
COLS = np.array(list(range(14)) + list(range(15, 18)))
PAD_NEG = -20.0   # pad channel fill (tanh -> -1; excluded from softmax sums)


def _perm_cols(w):
    # keep torch gate order (i,f,g,o): i,f,g contiguous so the chain-critical
    # tanh (i,f,g) can issue before the o quarter's matmuls finish
    return w


def _bf(x):
    return np.ascontiguousarray(np.asarray(x).astype(ml_dtypes.bfloat16))


def _f32(x):
    return np.ascontiguousarray(np.asarray(x).astype(np.float32))


def prep_weights(inp):
    w = {}
    w["Wi1R"] = _bf(np.concatenate([inp["Wi_w"].T, inp["Wi_b"][None, :]], 0))
    w["Wi2R"] = _bf(np.concatenate([inp["Wi2_w"].T * 0.5, inp["Wi2_b"][None, :]], 0))
    w["We1R"] = _bf(inp["We_w"].T * 0.5)
    w["We2R"] = _bf(inp["We2_w"].T * 0.5)
    w["WhR"] = _bf(inp["Wh_w"].T * 0.5)
    w["WxR"] = _bf(inp["Wx_w"].T * 0.5)
    w["Wxb"] = _bf(inp["Wx_b"][None, :])

    # ISO: sigmoid gates computed as tanh(x/2) -> pre-scale i,f,o cols by 0.5.
    # States are stored doubled (hS=2h, cS=2c), so weight blocks consuming
    # h/c/mid/din get an extra 0.5.
    ISO = np.concatenate([0.5 * np.ones(256), np.ones(128),
                          0.5 * np.ones(128)]).astype(np.float32)
    g1x = _perm_cols(inp["Wih1"].T) * ISO
    b1 = _perm_cols((inp["bih1"] + inp["bhh1"])[None, :]) * ISO
    w["G1XA"] = _bf(np.concatenate([g1x[0:9], b1], 0))   # +bias row (ones in x)
    w["G1XB"] = _bf(g1x[9:17])
    w["G1H"] = _bf(_perm_cols(inp["Whh1"].T) * ISO * 0.5)

    g2x = _perm_cols(inp["Wih2"].T) * ISO * 0.5
    b2 = _perm_cols((inp["bih2"] + inp["bhh2"])[None, :]) * ISO
    # stage-2 channel groups are chosen so x rows align with hT1 partitions:
    # group 0 = [h0..h63, label] (65 ch), group 1 = [h64..h127] (64 ch)
    w["G2XA"] = _bf(np.concatenate([g2x[0:64], g2x[128:129], b2], 0))  # +bias row
    w["G2XB"] = _bf(g2x[64:128])
    w["G2H"] = _bf(_perm_cols(inp["Whh2"].T) * ISO * 0.5)

    w["GdX"] = _bf(_perm_cols(inp["Wihd"].T) * ISO * 0.5)
    w["GdH"] = _bf(_perm_cols(inp["Whhd"].T) * ISO * 0.5)
    w["bdrow"] = _bf(_perm_cols((inp["bihd"] + inp["bhhd"])[None, :]) * ISO)

    w["vdup1"] = _bf(np.broadcast_to(inp["Vd_w"][0][None, :], (128, T)))
    w["vdup2"] = _bf(np.broadcast_to(inp["Vd2_w"][0][None, :], (128, T)))
    w["vdup3"] = _bf(np.broadcast_to(inp["V_w"][0][None, :], (128, H)))
    w["regw"] = _bf(inp["reg_w"][0][:, None] * 0.5)

    eye = np.eye(64, dtype=np.float32)
    w["I64dup"] = _f32(np.concatenate([eye, eye], 0))
    w["I64bf"] = _bf(np.concatenate([eye, eye], 0))
    w["I128bf"] = _bf(np.eye(128, dtype=np.float32))
    foldDup = (np.arange(128)[:, None] % 64 == np.arange(128)[None, :] % 64)
    w["foldDup"] = _f32(foldDup.astype(np.float32))
    w["onesrow"] = _bf(np.ones((1, 64), np.float32))
    return w


def prep_core_inputs(inp, core):
    b0, b1 = core * B, (core + 1) * B
    x = np.asarray(inp["input_p_q"])[b0:b1, :T, :][:, :, COLS]   # [64,64,17]
    lab = np.asarray(inp["label_p"])[b0:b1, :T]                  # [64,64]
    d = {}
    inpT = np.ones((65, NF * B), np.float32)
    inpT[:64] = x.transpose(1, 2, 0).reshape(64, NF * B)         # [t, (c,b)]
    d["inpT"] = _bf(inpT)
    ct = x.transpose(2, 1, 0).reshape(NF, T * B)                 # [c, (t,b)]
    d["inpCTA"] = _bf(ct[0:9])
    d["inpCTB"] = _bf(ct[9:17])
    d["labelT"] = _f32(lab.T * 2.0)                                    # [t, b]
    return d


DRAM_SPECS = {
    "inpT": ([65, NF * B], BF16), "inpCTA": ([9, T * B], BF16),
    "inpCTB": ([8, T * B], BF16), "labelT": ([T, B], F32),
    "Wi1R": ([65, 64], BF16), "Wi2R": ([65, 64], BF16),
    "We1R": ([256, 64], BF16), "We2R": ([256, 64], BF16),
    "WhR": ([256, 128], BF16), "WxR": ([128, 128], BF16), "Wxb": ([1, 128], BF16),
    "G1XA": ([10, 512], BF16), "G1XB": ([8, 512], BF16), "G1H": ([128, 512], BF16),
    "G2XA": ([66, 512], BF16), "G2XB": ([64, 512], BF16), "G2H": ([128, 512], BF16),
    "GdX": ([128, 512], BF16), "GdH": ([128, 512], BF16), "bdrow": ([1, 512], BF16),
    "vdup1": ([128, T], BF16), "vdup2": ([128, T], BF16), "vdup3": ([128, H], BF16),
    "regw": ([128, 1], BF16), "I64dup": ([128, 64], F32), "I64bf": ([128, 64], BF16),
    "I128bf": ([128, 128], BF16), "foldDup": ([128, 128], F32),
    "onesrow": ([1, 64], BF16),
}


def build_nc(num_devices=N_CORES, skip_score=False, skip_tail=False, only_stages=(1, 2, 3), split=(0.28, 0.64), split3=(0.25, 0.62)):
    nc = bacc.Bacc("TRN2", target_bir_lowering=False, debug=False,
                   num_devices=num_devices)
    dr = {}
    for name, (shape, dt) in DRAM_SPECS.items():
        dr[name] = nc.dram_tensor(name, shape, dt, kind="ExternalInput").ap()
    out_d = nc.dram_tensor("out", [B, 18], F32, kind="ExternalOutput").ap()

    with tile.TileContext(nc) as tc:
        # ---------- persistent SBUF ----------
        wpool = tc.alloc_tile_pool(name="wpool", bufs=1)
        sb = {}
        for name, (shape, dt) in DRAM_SPECS.items():
            if shape[0] > 128:
                assert shape[0] == 256
                for half, suf in ((0, "a"), (1, "b")):
                    key = name + suf
                    sb[key] = wpool.tile([128, shape[1]], dt, name=f"sb_{key}")
                    nc.sync.dma_start(sb[key][:], dr[name][128 * half:128 * (half + 1), :])
            else:
                sb[name] = wpool.tile(shape, dt, name=f"sb_{name}")
                nc.sync.dma_start(sb[name][:], dr[name][:])

        G2XBsh = wpool.tile([128, 512], BF16, name="G2XBsh")
        nc.sync.dma_start(G2XBsh[64:128, :], dr["G2XB"][:])
        X1 = wpool.tile([128, 9, T], BF16, name="X1")
        X2 = wpool.tile([128, 65, T], BF16, name="X2")
        WxF3 = wpool.tile([128, 32, H], BF16, name="WxF3")
        finB = wpool.tile([128, H, 32], BF16, name="finB")
        finT = wpool.tile([128, T, B], BF16, name="finT")    # stage-2 h, H-major
        hT1 = wpool.tile([128, T, B], BF16, name="hT1")      # stage-1 h, H-major
        labB = wpool.tile([65, T, B], BF16, name="labB")     # label at partition 64
        mid2T = wpool.tile([65, B, C2], BF16, name="mid2T")
        xA1pp = [wpool.tile([10, 64], BF16, name=f"xA1_{k}") for k in range(2)]
        xA2pp = [wpool.tile([66, 64], BF16, name=f"xA2_{k}") for k in range(2)]
        zeros128 = wpool.tile([128, 128], F32, name="zeros128")
        zerobf = wpool.tile([128, 64], BF16, name="zerobf")
        ones1 = wpool.tile([1, 64], BF16, name="ones1")
        outsb = wpool.tile([B, 18], F32, name="outsb")

        for k in range(2):
            nc.gpsimd.dma_start(xA1pp[k][9:10, :], dr["onesrow"][:])
            nc.gpsimd.dma_start(xA2pp[k][65:66, :], dr["onesrow"][:])
        nc.vector.memset(zerobf[:], 0.0)
        nc.vector.memset(zeros128[:], 0.0)
        nc.vector.memset(ones1[:], 1.0)
        nc.vector.memset(mid2T[64:65, :, :], 1.0)
        nc.vector.memset(X2[64:128, 64, :], PAD_NEG)
        nc.vector.memset(X1[64:128, 8, :], PAD_NEG)
        # label -> mid2T[t, b, 128] and labB[64, t, b]
        nc.gpsimd.dma_start(mid2T[0:64, :, 128:129], dr["labelT"][:])
        nc.gpsimd.dma_start(labB[64:65, :, :], dr["labelT"][:])

        if only_stages != (1, 2, 3):
            # profiling variants: init tiles a skipped stage would have written
            nc.vector.memset(finT[:], 0.1)
            nc.vector.memset(finB[:], 0.1)
            nc.vector.memset(hT1[:], 0.1)
            nc.vector.memset(mid2T[:], 0.1)
            nc.vector.memset(X2[:], 0.1)
            nc.vector.memset(X1[:], 0.1)
            nc.vector.memset(WxF3[:], 0.1)
            nc.vector.memset(outsb[:], 0.0)

        # ---------- X1 build ----------
        with tc.tile_pool(name="xb1", space="PSUM", bufs=1) as xb:
            x1ps = xb.tile([128, 9, T], F32, name="x1ps")
            for c in range(NF):
                g, ch = (0, c) if c < 9 else (1, c - 9)
                rows = slice(g * 64, g * 64 + 64)
                nc.tensor.matmul(x1ps[rows, ch, :],
                                 sb["inpT"][:, c * B:(c + 1) * B],
                                 sb["Wi1R"][:], start=True, stop=True)
            nc.vector.tensor_copy(X1[0:64, :, :], x1ps[0:64, :, :])
            nc.scalar.copy(X1[64:128, 0:8, :], x1ps[64:128, 0:8, :])

        # ================= helpers =================
        def lstm_hmajor(gps, cH_old, hdst, sp, pfx=""):
            """H-major doubled-state LSTM.  gps psum [128, 256] = [i|f|o|g]
            quarters, each [H=128, b=64].  Writes hS (bf16) into hdst and
            returns (cH_new f32, cTbf bf16)."""
            ta = sp.tile([128, 4, 64], F32, name=pfx + "ta", tag=pfx + "ta", bufs=2)
            # i,f,g first (feeds the c update); o separately (only needed by h)
            nc.scalar.activation(ta[:, 0:3, :], gps[:, 0:3, 0:64], AF.Tanh)
            nc.scalar.activation(ta[:, 3:4, :], gps[:, 3:4, 0:64], AF.Tanh)
            u = sp.tile([128, 64], F32, name=pfx + "u", tag=pfx + "u", bufs=2)
            v2 = sp.tile([128, 64], F32, name=pfx + "v2", tag=pfx + "v2", bufs=2)
            # u = (tanh(i/2)+1)*tanh(g) = 2*sig(i)*tanh(g)
            nc.vector.scalar_tensor_tensor(u[:], ta[:, 0, :], 1.0,
                                           ta[:, 2, :], op0=OP.add, op1=OP.mult)
            # v = (tanh(f/2)+1)*cS = 4*sig(f)*c
            nc.vector.scalar_tensor_tensor(v2[:], ta[:, 1, :], 1.0,
                                           cH_old[:], op0=OP.add, op1=OP.mult)
            cH = sp.tile([128, 64], F32, name=pfx + "cH", tag=pfx + "cH", bufs=2)
            # cS_new = v/2 + u = 2*c_new
            nc.vector.scalar_tensor_tensor(cH[:], v2[:], 0.5,
                                           u[:], op0=OP.mult, op1=OP.add)
            cTbf = sp.tile([128, 64], BF16, name=pfx + "cTb", tag=pfx + "cTb", bufs=2)
            nc.vector.tensor_copy(cTbf[:], cH[:])
            tcel = sp.tile([128, 64], F32, name=pfx + "tc", tag=pfx + "tc", bufs=2)
            nc.scalar.activation(tcel[:], cH[:], AF.Tanh, scale=0.5)
            # hS_new = (tanh(o/2)+1)*tanh(c) = 2*h_new
            nc.vector.scalar_tensor_tensor(hdst, ta[:, 3, :], 1.0,
                                           tcel[:], op0=OP.add, op1=OP.mult)
            return cH, cTbf

        def softmax_nomax(score, pool, ppool, nch, ptag="tps", pbufs=3,
                          want_a=True, exdt=F32):
            # score pad slots (if any) must already be ~-30 so exp ~ 0;
            # accum_out fuses the per-partition sum into the exp pass.
            ex = pool.tile([128, nch], exdt, name="ex", tag="sm_ex", bufs=2)
            zs = pool.tile([128, 1], F32, name="zs", tag="sm_zs", bufs=2)
            nc.scalar.activation(ex[:], score[:], AF.Exp, accum_out=zs[:])
            zps = ppool.tile([128, 1], F32, name="zps", tag=ptag, bufs=pbufs)
            nc.tensor.matmul(zps[:], sb["foldDup"][:], zs[:], start=True, stop=True)
            zr = pool.tile([128, 1], F32, name="zr", tag="sm_zr", bufs=2)
            nc.vector.reciprocal(zr[:], zps[:])
            if not want_a:
                return ex, zr
            a = pool.tile([128, nch], BF16, name="a", tag="sm_a", bufs=2)
            nc.vector.tensor_scalar_mul(a[:], ex[:], zr[:])
            return a

        def tree_to(dst, src, pool, tag, nch, ntau, single_cut=0, eng=None):
            """sum src [128, nch, ntau] over tau into dst [128, nch] slice."""
            if eng is None:
                eng = nc.vector
            if eng is not nc.vector:
                single_cut = 0   # Pool tensor_reduce can't do innermost-axis
            cur, n, lvl = src, ntau, 0
            while n > max(2, single_cut):
                n //= 2
                nxt = pool.tile([128, nch, n], BF16, name=f"{tag}_{lvl}",
                                tag=f"{tag}_{lvl}", bufs=1)
                eng.tensor_add(nxt[:], cur[:, :, 0:n], cur[:, :, n:2 * n])
                cur, lvl = nxt, lvl + 1
            if n > 2:
                with nc.allow_low_precision(reason="tiny bf16 tau-reduce"):
                    nc.vector.tensor_reduce(dst, cur[:], AX.X, OP.add)
            else:
                eng.tensor_add(dst.unsqueeze(-1), cur[:, :, 0:1], cur[:, :, 1:2])

        def score_chunked(Xs, esb, vdup, nch, ntau, sp, tag, pad_neg=False,
                          nchunks=2):
            """returns score [128, nch] bf16; chunks over ch for engine overlap."""
            score = sp.tile([128, nch], BF16, name="score", tag=f"{tag}_score",
                            bufs=2)
            if nchunks == 1:
                bounds = ((0, nch),)
            elif isinstance(nchunks, tuple):
                cuts = [0] + [max(1, min(nch - 1, int(round(nch * f)))) for f in nchunks] + [nch]
                bounds = tuple((cuts[i], cuts[i + 1]) for i in range(len(cuts) - 1))
            elif isinstance(nchunks, float):
                cut = max(1, min(nch - 1, int(round(nch * nchunks))))
                bounds = ((0, cut), (cut, nch))
            elif nchunks == 2:
                half = (nch + 1) // 2
                bounds = ((0, half), (half, nch))
            else:
                q = max(1, nch // nchunks)
                cuts = list(range(0, nch, q))
                bounds = tuple((lo, min(lo + q, nch)) for lo in cuts)
            for ci, (lo, hi) in enumerate(bounds):
                w = hi - lo
                scA = sp.tile([128, w, ntau], BF16, name="scA",
                              tag=f"{tag}_scA{lo}", bufs=1)
                nc.vector.tensor_add(scA[:], Xs[:, lo:hi, :],
                                     esb[:].unsqueeze(1).broadcast_to([128, w, ntau]))
                scT = sp.tile([128, w, ntau], BF16, name="scT",
                              tag=f"{tag}_scT{lo}", bufs=1)
                nc.scalar.activation(scT[:], scA[:], AF.Tanh)
                scM = sp.tile([128, w, ntau], BF16, name="scM",
                              tag=f"{tag}_scM{lo}", bufs=1)
                # the first chunk's result isn't needed until the exp, so its
                # mul+tree can run on the otherwise-idle Pool engine
                eng = nc.gpsimd if (ci == 0 and len(bounds) > 1) else nc.vector
                eng.tensor_mul(scM[:], scT[:],
                               vdup[:].unsqueeze(1).broadcast_to([128, w, ntau]))
                tree_to(score[:, lo:hi], scM, sp, f"{tag}_tr{lo}", w, ntau,
                        single_cut=16, eng=eng)
            if pad_neg:
                # kill the (g=1, ch=nch-1) pad slot before exp
                nc.vector.memset(score[64:128, nch - 1:nch], -30.0)
            return score

        # ================= encoder step =================
        def enc_step(t, stage, sp, pp, st):
            if stage == 1:
                Xs, vdup, WeRa, WeRb = X1, sb["vdup1"], sb["We1Ra"], sb["We1Rb"]
                nch = 9
                GH, GXA, GXB = sb["G1H"], sb["G1XA"], sb["G1XB"]
            else:
                Xs, vdup, WeRa, WeRb = X2, sb["vdup2"], sb["We2Ra"], sb["We2Rb"]
                nch = 65
                GH, GXA, GXB = sb["G2H"], sb["G2XA"], G2XBsh
            hT_old, cT_old, cH_old = st["hT"], st["cT"], st["cH"]
            tpsb = 2

            # e = [h;c] @ We  (b-major psum); c-part first (it's ready earlier)
            eps = pp.tile([128, T], F32, name="eps", tag="eps", bufs=1)
            for gb in (0, 64):
                o = eps[gb:gb + 64, :]
                nc.tensor.matmul(o, cT_old[:], WeRb[:], start=True, stop=False)
                nc.tensor.matmul(o, hT_old[:], WeRa[:], start=False, stop=True)
            esb = sp.tile([128, T], BF16, name="esb", tag="esb", bufs=2)
            nc.vector.tensor_copy(esb[:], eps[:])
            for fn in st.pop("defer", []):
                fn()

            # one bank per gate quarter: a start=True only zeroes its own bank
            gps = pp.tile([128, 4, 512], F32, name="gps", tag="gps", bufs=1)
            for q in range(4):
                nc.tensor.matmul(gps[:, q, 0:64], GH[:, 128 * q:128 * (q + 1)],
                                 hT_old[:], start=True, stop=False)

            if skip_score:
                score = sp.tile([128, nch], BF16, name="score", tag="e_score", bufs=2)
                nc.vector.memset(score[:], 0.1)
            else:
                score = score_chunked(Xs, esb, vdup, nch, T, sp, "e", pad_neg=True,
                                      nchunks=1 if stage == 1 else split)
            a = softmax_nomax(score, sp, pp, nch, pbufs=tpsb, exdt=BF16)

            if stage == 1:
                aTA = pp.tile([9, 64], BF16, name="aTA", tag="tps", bufs=tpsb)
                nc.tensor.transpose(aTA[:], a[0:64, 0:9], sb["I64bf"][0:64, :])
                aTB = pp.tile([8, 64], BF16, name="aTB", tag="tps", bufs=tpsb)
                nc.tensor.transpose(aTB[:], a[64:128, 0:8], sb["I64bf"][64:128, :])
                xB = sp.tile([8, 64], BF16, name="x1B", tag="xB", bufs=2)
                nc.vector.tensor_mul(xB[:], sb["inpCTB"][:, t * B:(t + 1) * B], aTB[:])
                xA = xA1pp[t % 2]
                nc.vector.tensor_mul(xA[0:9, :],
                                     sb["inpCTA"][:, t * B:(t + 1) * B], aTA[:])
            else:
                # group 0 = [h0..h63, label] at partitions 0..64,
                # group 1 = [h64..h127] at partitions 64..127 (psum base 64)
                aTA = pp.tile([65, 64], BF16, name="aTA", tag="tps", bufs=tpsb)
                nc.tensor.transpose(aTA[:], a[0:64, 0:65], sb["I64bf"][0:64, :])
                aTB = pp.tile([64, 64], BF16, name="aTB", tag="tps", bufs=tpsb)
                nc.tensor.transpose(aTB[:], a[64:128, 0:64], sb["I64bf"][64:128, :])
                xB = sp.tile([128, 64], BF16, name="x2B", tag="xB", bufs=2)
                nc.vector.tensor_mul(xB[64:128, :], hT1[64:128, t, :], aTB[0:64, :])
                xA = xA2pp[t % 2]
                nc.vector.tensor_mul(xA[0:64, :], hT1[0:64, t, :], aTA[0:64, :])
                nc.vector.tensor_mul(xA[64:65, :], labB[64:65, t, :], aTA[64:65, :])

            # late gate parts join the open per-bank groups; GXB first (xB is
            # the first x-mul to finish, so PE overlaps the remaining muls)
            for q in range(4):
                o = gps[:, q, 0:64]
                if stage == 1:
                    nc.tensor.matmul(o, GXB[:, 128 * q:128 * (q + 1)], xB[:],
                                     start=False, stop=False, skip_group_check=True)
                else:
                    nc.tensor.matmul(o, GXB[64:128, 128 * q:128 * (q + 1)],
                                     xB[64:128, :], start=False, stop=False,
                                     skip_group_check=True)
            for q in range(4):
                nc.tensor.matmul(gps[:, q, 0:64], GXA[:, 128 * q:128 * (q + 1)],
                                 xA[:], start=False, stop=True,
                                 skip_group_check=True)

            hdst = hT1[:, t, :] if stage == 1 else finT[:, t, :]
            cH, cTbf = lstm_hmajor(gps, cH_old, hdst, sp)
            st["hT"], st["cT"], st["cH"] = hdst, cTbf, cH

            # ---- off-critical-path per-step outputs; the ACT copies are
            # deferred past the next step's esb so they don't delay it ----
            if stage == 1:
                # mid2T[t] = h^T (b-major) for the X2 build
                def _defer(t=t, hdst=hdst):
                    hBps = pp.tile([64, 128], BF16, name="hBps", tag="hBps", bufs=1)
                    nc.tensor.transpose(hBps[:], hdst, sb["I128bf"][:])
                    hbf = sp.tile([64, 128], BF16, name="hbf", tag="hbf", bufs=2)
                    nc.scalar.copy(hbf[:], hBps[:])
                    if t % 2 == 0:
                        nc.sync.dma_start(mid2T[t:t + 1, :, 0:128], hbf[:])
                    else:
                        nc.gpsimd.dma_start(mid2T[t:t + 1, :, 0:128], hbf[:])
                st["defer"] = [_defer]
            else:
                g, sl = divmod(t, 32)
                rows = slice(g * 64, g * 64 + 64)
                # finB[(g,b), H, sl] = h^T  (transpose lands at base 0; the
                # copy shifts it to the group's partition block)
                def _defer(rows=rows, sl=sl, hdst=hdst):
                    fps = pp.tile([64, 128], BF16, name="fps", tag="fw", bufs=1)
                    nc.tensor.transpose(fps[:], hdst, sb["I128bf"][:])
                    wxps = pp.tile([128, 128], F32, name="wxps", tag="fw", bufs=1)
                    nc.tensor.matmul(wxps[rows, :], hdst, sb["WxR"][:],
                                     start=True, stop=True)
                    nc.scalar.copy(finB[rows, :, sl].unsqueeze(-1),
                                   fps[:].unsqueeze(-1))
                    nc.scalar.copy(WxF3[rows, sl, :], wxps[rows, :])
                st["defer"] = [_defer]

        # ---------- stage 1 ----------
        with tc.tile_pool(name="s1sp", bufs=2) as sp, \
             tc.tile_pool(name="s1pp", space="PSUM", bufs=2) as pp:
            st = {"hT": zerobf, "cT": zerobf, "cH": zeros128[:, 0:64]}
            for t in range(T if 1 in only_stages else 0):
                enc_step(t, 1, sp, pp, st)
            for fn in st.pop("defer", []):
                fn()

        # ---------- X2 build ----------
        with tc.tile_pool(name="xb2", space="PSUM", bufs=2) as xb2:
            for r in range(4):
                x2ps = xb2.tile([128, 16, T], F32, name="x2ps", tag="x2ps", bufs=2)
                for k in range(16):
                    ch = r * 16 + k
                    nc.tensor.matmul(x2ps[0:64, k, :], mid2T[:, :, ch],
                                     sb["Wi2R"][:], start=True, stop=True)
                    nc.tensor.matmul(x2ps[64:128, k, :], mid2T[:, :, 64 + ch],
                                     sb["Wi2R"][:], start=True, stop=True)
                nc.vector.tensor_copy(X2[:, r * 16:(r + 1) * 16, :], x2ps[:])
            x2ps2 = xb2.tile([64, T], F32, name="x2ps2", tag="x2ps2", bufs=1)
            nc.tensor.matmul(x2ps2[:], mid2T[:, :, 128], sb["Wi2R"][:],
                             start=True, stop=True)
            nc.vector.tensor_copy(X2[0:64, 64, :], x2ps2[:])

        # ---------- stage 2 ----------
        with tc.tile_pool(name="s2sp", bufs=2) as sp, \
             tc.tile_pool(name="s2pp", space="PSUM", bufs=2) as pp:
            st = {"hT": zerobf, "cT": zerobf, "cH": zeros128[:, 0:64]}
            for t in range(T if 2 in only_stages else 0):
                enc_step(t, 2, sp, pp, st)
            for fn in st.pop("defer", []):
                fn()

        # ---------- stage 3 ----------
        with tc.tile_pool(name="s3sp", bufs=2) as sp, \
             tc.tile_pool(name="s3pp", space="PSUM", bufs=2) as pp:
            outps = pp.tile([64, 18], F32, name="outps", bufs=1) if 3 in only_stages else None
            st = {"hT": zerobf, "cT": zerobf, "cH": zeros128[:, 0:64]}
            for t in range(TD if 3 in only_stages else 0):
                hT_old, cT_old, cH_old = st["hT"], st["cT"], st["cH"]
                eps = pp.tile([128, H], F32, name="e3ps", tag="eps3", bufs=1)
                for gb in (0, 64):
                    o = eps[gb:gb + 64, :]
                    nc.tensor.matmul(o, ones1[:], sb["Wxb"][:], start=True, stop=False)
                    nc.tensor.matmul(o, cT_old[:], sb["WhRb"][:],
                                     start=False, stop=False)
                    nc.tensor.matmul(o, hT_old[:], sb["WhRa"][:],
                                     start=False, stop=True)
                esb = sp.tile([128, H], BF16, name="e3sb", tag="esb3", bufs=2)
                nc.vector.tensor_copy(esb[:], eps[:])

                gps = pp.tile([128, 4, 512], F32, name="g3ps", tag="g3ps", bufs=1)
                for q in range(4):
                    o = gps[:, q, 0:64]
                    nc.tensor.matmul(o, sb["bdrow"][:, 128 * q:128 * (q + 1)],
                                     ones1[:], start=True, stop=False)
                    nc.tensor.matmul(o, sb["GdH"][:, 128 * q:128 * (q + 1)],
                                     hT_old[:], start=False, stop=False)

                if skip_score:
                    score = sp.tile([128, 32], BF16, name="score", tag="d_score", bufs=2)
                    nc.vector.memset(score[:], 0.1)
                else:
                    score = score_chunked(WxF3, esb, sb["vdup3"], 32, H, sp, "d",
                                          nchunks=split3 if split3 is not None else split)
                ex, zr = softmax_nomax(score, sp, pp, 32, ptag="tps3", pbufs=2,
                                       want_a=False, exdt=BF16)

                # context from unnormalized ex (starts right after exp; the
                # fold/recip run in parallel), then scale uu by 1/Z
                # (per-partition) before the fold+transpose matmul.
                ym = sp.tile([128, H, 32], BF16, name="ym", tag="ym", bufs=1)
                nc.vector.tensor_mul(ym[:, 0:96, :], finB[:, 0:96, :],
                                     ex[:].unsqueeze(1).broadcast_to([128, 96, 32]))
                nc.gpsimd.tensor_mul(ym[:, 96:128, :], finB[:, 96:128, :],
                                     ex[:].unsqueeze(1).broadcast_to([128, 32, 32]))
                uu0 = sp.tile([128, H], BF16, name="uu0", tag="uu0", bufs=2)
                tree_to(uu0[:, 0:96], ym[:, 0:96, :], sp, "ctr", 96, 32, single_cut=0)
                tree_to(uu0[:, 96:128], ym[:, 96:128, :], sp, "ctrp", 32, 32,
                        single_cut=0, eng=nc.gpsimd)
                uu = sp.tile([128, H], BF16, name="uu", tag="uu", bufs=2)
                nc.vector.tensor_scalar_mul(uu[:], uu0[:], zr[:])
                dinps = pp.tile([128, 64], F32, name="dinps", tag="tps3", bufs=2)
                nc.tensor.matmul(dinps[:], uu[:], sb["I64bf"][:],
                                 start=True, stop=True)
                dinT = sp.tile([128, 64], BF16, name="dinT", tag="dinT", bufs=2)
                nc.vector.tensor_copy(dinT[:], dinps[:])

                # late gate part: din joins the open per-bank groups
                for q in range(4):
                    nc.tensor.matmul(gps[:, q, 0:64],
                                     sb["GdX"][:, 128 * q:128 * (q + 1)],
                                     dinT[:], start=False, stop=True,
                                     skip_group_check=True)

                h3 = sp.tile([128, 64], BF16, name="h3", tag="h3", bufs=2)
                cH, cTbf = lstm_hmajor(gps, cH_old, h3[:], sp, pfx="3")
                st["hT"], st["cT"], st["cH"] = h3, cTbf, cH

                if t >= TD - 18:
                    j = t - (TD - 18)
                    nc.tensor.matmul(outps[:, j:j + 1], h3[:], sb["regw"][:],
                                     start=True, stop=True)

            if 3 in only_stages:
                nc.vector.tensor_copy(outsb[:], outps[:])
            nc.sync.dma_start(out_d[:], outsb[:])

        wpool.release()

    nc.compile()
    return nc


_NC_CACHE = {}


def kernel(**inputs):
    if "nc" not in _NC_CACHE:
        _NC_CACHE["nc"] = build_nc()
    nc = _NC_CACHE["nc"]
    w = prep_weights({k: np.asarray(v) for k, v in inputs.items()})
    in_maps = []
    for core in range(N_CORES):
        m = dict(w)
        m.update(prep_core_inputs(inputs, core))
        in_maps.append(m)
    res = run_bass_kernel_spmd(nc, in_maps, list(range(N_CORES)))
    out = np.concatenate([res.results[c]["out"] for c in range(N_CORES)], axis=0)
    out = out + np.asarray(inputs["reg_b"])[0]
    return out.astype(np.float32)


# revision 33
# speedup vs baseline: 1.0065x; 1.0033x over previous
# DSTP-RNN Trainium2 kernel: 8-core pure data parallel (batch 512 -> 64/core).
#
# Layout summary:
#  - "Score" tensors are b-major: partitions = (g, b) with g a channel-group
#    split, b = 64 local batch rows; free dims = (ch, tau).
#  - LSTM runs H-major: gates land in PSUM [H=128, b=64] via per-quarter
#    matmuls (stationary = weight slices), so h/c are produced directly in
#    the [H, b] layout the next step's matmuls consume - no transposes on
#    the recurrent chain.  States are doubled (hS=2h, cS=2c) and sigmoids
#    are computed as tanh(x/2) with pre-scaled weights.
#  - All matmul operands are bf16 (1 cycle/row); elementwise stays f32.
#  - Gate accumulation is split: bias+h parts issue right after the e-matmul
#    (overlapping the attention score), x parts join at the end.
#  - Off-critical-path copies (finB, WxF, mid2T staging) go to ACT/Pool.
import numpy as np
import ml_dtypes

import concourse.bacc as bacc
import concourse.mybir as mybir
import concourse.tile as tile
from concourse.bass_utils import run_bass_kernel_spmd

F32 = mybir.dt.float32
BF16 = mybir.dt.bfloat16
AX = mybir.AxisListType
OP = mybir.AluOpType
AF = mybir.ActivationFunctionType

N_CORES = 8
B = 64      # batch per core
T = 64      # encoder length
H = 128
TD = 24     # decoder steps (T_DEC + 6)
NF = 17     # driving series count
C2 = 129    # stage-2 channels (H + label)
COLS = np.array(list(range(14)) + list(range(15, 18)))
PAD_NEG = -20.0   # pad channel fill (tanh -> -1; excluded from softmax sums)


def _perm_cols(w):
    # keep torch gate order (i,f,g,o): i,f,g contiguous so the chain-critical
    # tanh (i,f,g) can issue before the o quarter's matmuls finish
    return w


def _bf(x):
    return np.ascontiguousarray(np.asarray(x).astype(ml_dtypes.bfloat16))


def _f32(x):
    return np.ascontiguousarray(np.asarray(x).astype(np.float32))


def prep_weights(inp):
    w = {}
    w["Wi1R"] = _bf(np.concatenate([inp["Wi_w"].T, inp["Wi_b"][None, :]], 0))
    w["Wi2R"] = _bf(np.concatenate([inp["Wi2_w"].T * 0.5, inp["Wi2_b"][None, :]], 0))
    w["We1R"] = _bf(inp["We_w"].T * 0.5)
    w["We2R"] = _bf(inp["We2_w"].T * 0.5)
    w["WhR"] = _bf(inp["Wh_w"].T * 0.5)
    w["WxR"] = _bf(inp["Wx_w"].T * 0.5)
    w["Wxb"] = _bf(inp["Wx_b"][None, :])

    # ISO: sigmoid gates computed as tanh(x/2) -> pre-scale i,f,o cols by 0.5.
    # States are stored doubled (hS=2h, cS=2c), so weight blocks consuming
    # h/c/mid/din get an extra 0.5.
    ISO = np.concatenate([0.5 * np.ones(256), np.ones(128),
                          0.5 * np.ones(128)]).astype(np.float32)
    g1x = _perm_cols(inp["Wih1"].T) * ISO
    b1 = _perm_cols((inp["bih1"] + inp["bhh1"])[None, :]) * ISO
    w["G1XA"] = _bf(np.concatenate([g1x[0:9], b1], 0))   # +bias row (ones in x)
    w["G1XB"] = _bf(g1x[9:17])
    w["G1H"] = _bf(_perm_cols(inp["Whh1"].T) * ISO * 0.5)

    g2x = _perm_cols(inp["Wih2"].T) * ISO * 0.5
    b2 = _perm_cols((inp["bih2"] + inp["bhh2"])[None, :]) * ISO
    # stage-2 channel groups are chosen so x rows align with hT1 partitions:
    # group 0 = [h0..h63, label] (65 ch), group 1 = [h64..h127] (64 ch)
    w["G2XA"] = _bf(np.concatenate([g2x[0:64], g2x[128:129], b2], 0))  # +bias row
    w["G2XB"] = _bf(g2x[64:128])
    w["G2H"] = _bf(_perm_cols(inp["Whh2"].T) * ISO * 0.5)

    w["GdX"] = _bf(_perm_cols(inp["Wihd"].T) * ISO * 0.5)
    w["GdH"] = _bf(_perm_cols(inp["Whhd"].T) * ISO * 0.5)
    w["bdrow"] = _bf(_perm_cols((inp["bihd"] + inp["bhhd"])[None, :]) * ISO)

    w["vdup1"] = _bf(np.broadcast_to(inp["Vd_w"][0][None, :], (128, T)))
    w["vdup2"] = _bf(np.broadcast_to(inp["Vd2_w"][0][None, :], (128, T)))
    w["vdup3"] = _bf(np.broadcast_to(inp["V_w"][0][None, :], (128, H)))
    w["regw"] = _bf(inp["reg_w"][0][:, None] * 0.5)

    eye = np.eye(64, dtype=np.float32)
    w["I64dup"] = _f32(np.concatenate([eye, eye], 0))
    w["I64bf"] = _bf(np.concatenate([eye, eye], 0))
    w["I128bf"] = _bf(np.eye(128, dtype=np.float32))
    foldDup = (np.arange(128)[:, None] % 64 == np.arange(128)[None, :] % 64)
    w["foldDup"] = _f32(foldDup.astype(np.float32))
    w["onesrow"] = _bf(np.ones((1, 64), np.float32))
    return w


def prep_core_inputs(inp, core):
    b0, b1 = core * B, (core + 1) * B
    x = np.asarray(inp["input_p_q"])[b0:b1, :T, :][:, :, COLS]   # [64,64,17]
    lab = np.asarray(inp["label_p"])[b0:b1, :T]                  # [64,64]
    d = {}
    inpT = np.ones((65, NF * B), np.float32)
    inpT[:64] = x.transpose(1, 2, 0).reshape(64, NF * B)         # [t, (c,b)]
    d["inpT"] = _bf(inpT)
    ct = x.transpose(2, 1, 0).reshape(NF, T * B)                 # [c, (t,b)]
    d["inpCTA"] = _bf(ct[0:9])
    d["inpCTB"] = _bf(ct[9:17])
    d["labelT"] = _f32(lab.T * 2.0)                                    # [t, b]
    return d


DRAM_SPECS = {
    "inpT": ([65, NF * B], BF16), "inpCTA": ([9, T * B], BF16),
    "inpCTB": ([8, T * B], BF16), "labelT": ([T, B], F32),
    "Wi1R": ([65, 64], BF16), "Wi2R": ([65, 64], BF16),
    "We1R": ([256, 64], BF16), "We2R": ([256, 64], BF16),
    "WhR": ([256, 128], BF16), "WxR": ([128, 128], BF16), "Wxb": ([1, 128], BF16),
    "G1XA": ([10, 512], BF16), "G1XB": ([8, 512], BF16), "G1H": ([128, 512], BF16),
    "G2XA": ([66, 512], BF16), "G2XB": ([64, 512], BF16), "G2H": ([128, 512], BF16),
    "GdX": ([128, 512], BF16), "GdH": ([128, 512], BF16), "bdrow": ([1, 512], BF16),
    "vdup1": ([128, T], BF16), "vdup2": ([128, T], BF16), "vdup3": ([128, H], BF16),
    "regw": ([128, 1], BF16), "I64dup": ([128, 64], F32), "I64bf": ([128, 64], BF16),
    "I128bf": ([128, 128], BF16), "foldDup": ([128, 128], F32),
    "onesrow": ([1, 64], BF16),
}


def build_nc(num_devices=N_CORES, skip_score=False, skip_tail=False, only_stages=(1, 2, 3), split=(0.28, 0.64), split3=(0.25, 0.62)):
    nc = bacc.Bacc("TRN2", target_bir_lowering=False, debug=False,
                   num_devices=num_devices)
    dr = {}
    for name, (shape, dt) in DRAM_SPECS.items():
        dr[name] = nc.dram_tensor(name, shape, dt, kind="ExternalInput").ap()
    out_d = nc.dram_tensor("out", [B, 18], F32, kind="ExternalOutput").ap()

    with tile.TileContext(nc) as tc:
        # ---------- persistent SBUF ----------
        wpool = tc.alloc_tile_pool(name="wpool", bufs=1)
        sb = {}
        for name, (shape, dt) in DRAM_SPECS.items():
            if shape[0] > 128:
                assert shape[0] == 256
                for half, suf in ((0, "a"), (1, "b")):
                    key = name + suf
                    sb[key] = wpool.tile([128, shape[1]], dt, name=f"sb_{key}")
                    nc.sync.dma_start(sb[key][:], dr[name][128 * half:128 * (half + 1), :])
            else:
                sb[name] = wpool.tile(shape, dt, name=f"sb_{name}")
                nc.sync.dma_start(sb[name][:], dr[name][:])

        G2XBsh = wpool.tile([128, 512], BF16, name="G2XBsh")
        nc.sync.dma_start(G2XBsh[64:128, :], dr["G2XB"][:])
        X1 = wpool.tile([128, 9, T], BF16, name="X1")
        X2 = wpool.tile([128, 65, T], BF16, name="X2")
        WxF3 = wpool.tile([128, 32, H], BF16, name="WxF3")
        finB = wpool.tile([128, H, 32], BF16, name="finB")
        finT = wpool.tile([128, T, B], BF16, name="finT")    # stage-2 h, H-major
        hT1 = wpool.tile([128, T, B], BF16, name="hT1")      # stage-1 h, H-major
        labB = wpool.tile([65, T, B], BF16, name="labB")     # label at partition 64
        mid2T = wpool.tile([65, B, C2], BF16, name="mid2T")
        xA1pp = [wpool.tile([10, 64], BF16, name=f"xA1_{k}") for k in range(2)]
        xA2pp = [wpool.tile([66, 64], BF16, name=f"xA2_{k}") for k in range(2)]
        zeros128 = wpool.tile([128, 128], F32, name="zeros128")
        zerobf = wpool.tile([128, 64], BF16, name="zerobf")
        ones1 = wpool.tile([1, 64], BF16, name="ones1")
        outsb = wpool.tile([B, 18], F32, name="outsb")

        for k in range(2):
            nc.gpsimd.dma_start(xA1pp[k][9:10, :], dr["onesrow"][:])
            nc.gpsimd.dma_start(xA2pp[k][65:66, :], dr["onesrow"][:])
        nc.vector.memset(zerobf[:], 0.0)
        nc.vector.memset(zeros128[:], 0.0)
        nc.vector.memset(ones1[:], 1.0)
        nc.vector.memset(mid2T[64:65, :, :], 1.0)
        nc.vector.memset(X2[64:128, 64, :], PAD_NEG)
        nc.vector.memset(X1[64:128, 8, :], PAD_NEG)
        # label -> mid2T[t, b, 128] and labB[64, t, b]
        nc.gpsimd.dma_start(mid2T[0:64, :, 128:129], dr["labelT"][:])
        nc.gpsimd.dma_start(labB[64:65, :, :], dr["labelT"][:])

        if only_stages != (1, 2, 3):
            # profiling variants: init tiles a skipped stage would have written
            nc.vector.memset(finT[:], 0.1)
            nc.vector.memset(finB[:], 0.1)
            nc.vector.memset(hT1[:], 0.1)
            nc.vector.memset(mid2T[:], 0.1)
            nc.vector.memset(X2[:], 0.1)
            nc.vector.memset(X1[:], 0.1)
            nc.vector.memset(WxF3[:], 0.1)
            nc.vector.memset(outsb[:], 0.0)

        # ---------- X1 build ----------
        with tc.tile_pool(name="xb1", space="PSUM", bufs=1) as xb:
            x1ps = xb.tile([128, 9, T], F32, name="x1ps")
            for c in range(NF):
                g, ch = (0, c) if c < 9 else (1, c - 9)
                rows = slice(g * 64, g * 64 + 64)
                nc.tensor.matmul(x1ps[rows, ch, :],
                                 sb["inpT"][:, c * B:(c + 1) * B],
                                 sb["Wi1R"][:], start=True, stop=True)
            nc.vector.tensor_copy(X1[0:64, :, :], x1ps[0:64, :, :])
            nc.scalar.copy(X1[64:128, 0:8, :], x1ps[64:128, 0:8, :])

        # ================= helpers =================
        def lstm_hmajor(gps, cH_old, hdst, sp, pfx=""):
            """H-major doubled-state LSTM.  gps psum [128, 256] = [i|f|o|g]
            quarters, each [H=128, b=64].  Writes hS (bf16) into hdst and
            returns (cH_new f32, cTbf bf16)."""
            ta = sp.tile([128, 4, 64], F32, name=pfx + "ta", tag=pfx + "ta", bufs=2)
            # i,f,g first (feeds the c update); o separately (only needed by h)
            nc.scalar.activation(ta[:, 0:3, :], gps[:, 0:3, 0:64], AF.Tanh)
            nc.scalar.activation(ta[:, 3:4, :], gps[:, 3:4, 0:64], AF.Tanh)
            u = sp.tile([128, 64], F32, name=pfx + "u", tag=pfx + "u", bufs=2)
            v2 = sp.tile([128, 64], F32, name=pfx + "v2", tag=pfx + "v2", bufs=2)
            # u = (tanh(i/2)+1)*tanh(g) = 2*sig(i)*tanh(g)
            nc.vector.scalar_tensor_tensor(u[:], ta[:, 0, :], 1.0,
                                           ta[:, 2, :], op0=OP.add, op1=OP.mult)
            # v = (tanh(f/2)+1)*cS = 4*sig(f)*c
            nc.vector.scalar_tensor_tensor(v2[:], ta[:, 1, :], 1.0,
                                           cH_old[:], op0=OP.add, op1=OP.mult)
            cH = sp.tile([128, 64], F32, name=pfx + "cH", tag=pfx + "cH", bufs=2)
            # cS_new = v/2 + u = 2*c_new
            nc.vector.scalar_tensor_tensor(cH[:], v2[:], 0.5,
                                           u[:], op0=OP.mult, op1=OP.add)
            cTbf = sp.tile([128, 64], BF16, name=pfx + "cTb", tag=pfx + "cTb", bufs=2)
            nc.vector.tensor_copy(cTbf[:], cH[:])
            tcel = sp.tile([128, 64], F32, name=pfx + "tc", tag=pfx + "tc", bufs=2)
            nc.scalar.activation(tcel[:], cH[:], AF.Tanh, scale=0.5)
            # hS_new = (tanh(o/2)+1)*tanh(c) = 2*h_new
            nc.vector.scalar_tensor_tensor(hdst, ta[:, 3, :], 1.0,
                                           tcel[:], op0=OP.add, op1=OP.mult)
            return cH, cTbf

        def softmax_nomax(score, pool, ppool, nch, ptag="tps", pbufs=3,
                          want_a=True, exdt=F32):
            # score pad slots (if any) must already be ~-30 so exp ~ 0;
            # accum_out fuses the per-partition sum into the exp pass.
            ex = pool.tile([128, nch], exdt, name="ex", tag="sm_ex", bufs=2)
            zs = pool.tile([128, 1], F32, name="zs", tag="sm_zs", bufs=2)
            nc.scalar.activation(ex[:], score[:], AF.Exp, accum_out=zs[:])
            zps = ppool.tile([128, 1], F32, name="zps", tag=ptag, bufs=pbufs)
            nc.tensor.matmul(zps[:], sb["foldDup"][:], zs[:], start=True, stop=True)
            zr = pool.tile([128, 1], F32, name="zr", tag="sm_zr", bufs=2)
            nc.vector.reciprocal(zr[:], zps[:])
            if not want_a:
                return ex, zr
            a = pool.tile([128, nch], BF16, name="a", tag="sm_a", bufs=2)
            nc.vector.tensor_scalar_mul(a[:], ex[:], zr[:])
            return a

        def tree_to(dst, src, pool, tag, nch, ntau, single_cut=0, eng=None):
            """sum src [128, nch, ntau] over tau into dst [128, nch] slice."""
            if eng is None:
                eng = nc.vector
            if eng is not nc.vector:
                single_cut = 0   # Pool tensor_reduce can't do innermost-axis
            cur, n, lvl = src, ntau, 0
            while n > max(2, single_cut):
                n //= 2
                nxt = pool.tile([128, nch, n], BF16, name=f"{tag}_{lvl}",
                                tag=f"{tag}_{lvl}", bufs=1)
                eng.tensor_add(nxt[:], cur[:, :, 0:n], cur[:, :, n:2 * n])
                cur, lvl = nxt, lvl + 1
            if n > 2:
                with nc.allow_low_precision(reason="tiny bf16 tau-reduce"):
                    nc.vector.tensor_reduce(dst, cur[:], AX.X, OP.add)
            else:
                eng.tensor_add(dst.unsqueeze(-1), cur[:, :, 0:1], cur[:, :, 1:2])

        def score_chunked(Xs, esb, vdup, nch, ntau, sp, tag, pad_neg=False,
                          nchunks=2):
            """returns score [128, nch] bf16; chunks over ch for engine overlap."""
            score = sp.tile([128, nch], BF16, name="score", tag=f"{tag}_score",
                            bufs=2)
            if nchunks == 1:
                bounds = ((0, nch),)
            elif isinstance(nchunks, tuple):
                cuts = [0] + [max(1, min(nch - 1, int(round(nch * f)))) for f in nchunks] + [nch]
                bounds = tuple((cuts[i], cuts[i + 1]) for i in range(len(cuts) - 1))
            elif isinstance(nchunks, float):
                cut = max(1, min(nch - 1, int(round(nch * nchunks))))
                bounds = ((0, cut), (cut, nch))
            elif nchunks == 2:
                half = (nch + 1) // 2
                bounds = ((0, half), (half, nch))
            else:
                q = max(1, nch // nchunks)
                cuts = list(range(0, nch, q))
                bounds = tuple((lo, min(lo + q, nch)) for lo in cuts)
            for ci, (lo, hi) in enumerate(bounds):
                w = hi - lo
                scA = sp.tile([128, w, ntau], BF16, name="scA",
                              tag=f"{tag}_scA{lo}", bufs=1)
                nc.vector.tensor_add(scA[:], Xs[:, lo:hi, :],
                                     esb[:].unsqueeze(1).broadcast_to([128, w, ntau]))
                scT = sp.tile([128, w, ntau], BF16, name="scT",
                              tag=f"{tag}_scT{lo}", bufs=1)
                nc.scalar.activation(scT[:], scA[:], AF.Tanh)
                scM = sp.tile([128, w, ntau], BF16, name="scM",
                              tag=f"{tag}_scM{lo}", bufs=1)
                # the first chunk's result isn't needed until the exp, so its
                # mul+tree can run on the otherwise-idle Pool engine
                eng = nc.gpsimd if (ci == 0 and len(bounds) > 1) else nc.vector
                eng.tensor_mul(scM[:], scT[:],
                               vdup[:].unsqueeze(1).broadcast_to([128, w, ntau]))
                tree_to(score[:, lo:hi], scM, sp, f"{tag}_tr{lo}", w, ntau,
                        single_cut=16, eng=eng)
            if pad_neg:
                # kill the (g=1, ch=nch-1) pad slot before exp
                nc.vector.memset(score[64:128, nch - 1:nch], -30.0)
            return score

        # ================= encoder step =================
        def enc_step(t, stage, sp, pp, st):
            if stage == 1:
                Xs, vdup, WeRa, WeRb = X1, sb["vdup1"], sb["We1Ra"], sb["We1Rb"]
                nch = 9
                GH, GXA, GXB = sb["G1H"], sb["G1XA"], sb["G1XB"]
            else:
                Xs, vdup, WeRa, WeRb = X2, sb["vdup2"], sb["We2Ra"], sb["We2Rb"]
                nch = 65
                GH, GXA, GXB = sb["G2H"], sb["G2XA"], G2XBsh
            hT_old, cT_old, cH_old = st["hT"], st["cT"], st["cH"]
            tpsb = 2

            # e = [h;c] @ We  (b-major psum); c-part first (it's ready earlier)
            eps = pp.tile([128, T], F32, name="eps", tag="eps", bufs=1)
            for gb in (0, 64):
                o = eps[gb:gb + 64, :]
                nc.tensor.matmul(o, cT_old[:], WeRb[:], start=True, stop=False)
                nc.tensor.matmul(o, hT_old[:], WeRa[:], start=False, stop=True)
            esb = sp.tile([128, T], BF16, name="esb", tag="esb", bufs=2)
            nc.vector.tensor_copy(esb[:], eps[:])
            for fn in st.pop("defer", []):
                fn()

            # one bank per gate quarter: a start=True only zeroes its own bank
            gps = pp.tile([128, 4, 512], F32, name="gps", tag="gps", bufs=1)
            for q in range(4):
                nc.tensor.matmul(gps[:, q, 0:64], GH[:, 128 * q:128 * (q + 1)],
                                 hT_old[:], start=True, stop=False)

            if skip_score:
                score = sp.tile([128, nch], BF16, name="score", tag="e_score", bufs=2)
                nc.vector.memset(score[:], 0.1)
            else:
                score = score_chunked(Xs, esb, vdup, nch, T, sp, "e", pad_neg=True,
                                      nchunks=1 if stage == 1 else split)
            a = softmax_nomax(score, sp, pp, nch, pbufs=tpsb, exdt=BF16)

            if stage == 1:
                aTA = pp.tile([9, 64], BF16, name="aTA", tag="tps", bufs=tpsb)
                nc.tensor.transpose(aTA[:], a[0:64, 0:9], sb["I64bf"][0:64, :])
                aTB = pp.tile([8, 64], BF16, name="aTB", tag="tps", bufs=tpsb)
                nc.tensor.transpose(aTB[:], a[64:128, 0:8], sb["I64bf"][64:128, :])
                xB = sp.tile([8, 64], BF16, name="x1B", tag="xB", bufs=2)
                nc.vector.tensor_mul(xB[:], sb["inpCTB"][:, t * B:(t + 1) * B], aTB[:])
                xA = xA1pp[t % 2]
                nc.vector.tensor_mul(xA[0:9, :],
                                     sb["inpCTA"][:, t * B:(t + 1) * B], aTA[:])
            else:
                # group 0 = [h0..h63, label] at partitions 0..64,
                # group 1 = [h64..h127] at partitions 64..127 (psum base 64)
                aTA = pp.tile([65, 64], BF16, name="aTA", tag="tps", bufs=tpsb)
                nc.tensor.transpose(aTA[:], a[0:64, 0:65], sb["I64bf"][0:64, :])
                aTB = pp.tile([64, 64], BF16, name="aTB", tag="tps", bufs=tpsb)
                nc.tensor.transpose(aTB[:], a[64:128, 0:64], sb["I64bf"][64:128, :])
                xB = sp.tile([128, 64], BF16, name="x2B", tag="xB", bufs=2)
                nc.vector.tensor_mul(xB[64:128, :], hT1[64:128, t, :], aTB[0:64, :])
                xA = xA2pp[t % 2]
                nc.vector.tensor_mul(xA[0:64, :], hT1[0:64, t, :], aTA[0:64, :])
                nc.vector.tensor_mul(xA[64:65, :], labB[64:65, t, :], aTA[64:65, :])

            # late gate parts join the open per-bank groups; GXB first (xB is
            # the first x-mul to finish, so PE overlaps the remaining muls)
            for q in range(4):
                o = gps[:, q, 0:64]
                if stage == 1:
                    nc.tensor.matmul(o, GXB[:, 128 * q:128 * (q + 1)], xB[:],
                                     start=False, stop=False, skip_group_check=True)
                else:
                    nc.tensor.matmul(o, GXB[64:128, 128 * q:128 * (q + 1)],
                                     xB[64:128, :], start=False, stop=False,
                                     skip_group_check=True)
            for q in range(4):
                nc.tensor.matmul(gps[:, q, 0:64], GXA[:, 128 * q:128 * (q + 1)],
                                 xA[:], start=False, stop=True,
                                 skip_group_check=True)

            hdst = hT1[:, t, :] if stage == 1 else finT[:, t, :]
            cH, cTbf = lstm_hmajor(gps, cH_old, hdst, sp)
            st["hT"], st["cT"], st["cH"] = hdst, cTbf, cH

            # ---- off-critical-path per-step outputs; the ACT copies are
            # deferred past the next step's esb so they don't delay it ----
            if stage == 1:
                # mid2T[t] = h^T (b-major) for the X2 build
                def _defer(t=t, hdst=hdst):
                    hBps = pp.tile([64, 128], BF16, name="hBps", tag="hBps", bufs=1)
                    nc.tensor.transpose(hBps[:], hdst, sb["I128bf"][:])
                    hbf = sp.tile([64, 128], BF16, name="hbf", tag="hbf", bufs=2)
                    nc.scalar.copy(hbf[:], hBps[:])
                    if t % 2 == 0:
                        nc.sync.dma_start(mid2T[t:t + 1, :, 0:128], hbf[:])
                    else:
                        nc.gpsimd.dma_start(mid2T[t:t + 1, :, 0:128], hbf[:])
                st["defer"] = [_defer]
            else:
                g, sl = divmod(t, 32)
                rows = slice(g * 64, g * 64 + 64)
                # finB[(g,b), H, sl] = h^T  (transpose lands at base 0; the
                # copy shifts it to the group's partition block)
                def _defer(rows=rows, sl=sl, hdst=hdst):
                    fps = pp.tile([64, 128], BF16, name="fps", tag="fw", bufs=1)
                    nc.tensor.transpose(fps[:], hdst, sb["I128bf"][:])
                    wxps = pp.tile([128, 128], F32, name="wxps", tag="fw", bufs=1)
                    nc.tensor.matmul(wxps[rows, :], hdst, sb["WxR"][:],
                                     start=True, stop=True)
                    nc.scalar.copy(finB[rows, :, sl].unsqueeze(-1),
                                   fps[:].unsqueeze(-1))
                    nc.scalar.copy(WxF3[rows, sl, :], wxps[rows, :])
                st["defer"] = [_defer]

        # ---------- stage 1 ----------
        with tc.tile_pool(name="s1sp", bufs=2) as sp, \
             tc.tile_pool(name="s1pp", space="PSUM", bufs=2) as pp:
            st = {"hT": zerobf, "cT": zerobf, "cH": zeros128[:, 0:64]}
            for t in range(T if 1 in only_stages else 0):
                enc_step(t, 1, sp, pp, st)
            for fn in st.pop("defer", []):
                fn()

        # ---------- X2 build ----------
        with tc.tile_pool(name="xb2", space="PSUM", bufs=2) as xb2:
            for r in range(4):
                x2ps = xb2.tile([128, 16, T], F32, name="x2ps", tag="x2ps", bufs=2)
                for k in range(16):
                    ch = r * 16 + k
                    nc.tensor.matmul(x2ps[0:64, k, :], mid2T[:, :, ch],
                                     sb["Wi2R"][:], start=True, stop=True)
                    nc.tensor.matmul(x2ps[64:128, k, :], mid2T[:, :, 64 + ch],
                                     sb["Wi2R"][:], start=True, stop=True)
                nc.vector.tensor_copy(X2[:, r * 16:(r + 1) * 16, :], x2ps[:])
            x2ps2 = xb2.tile([64, T], F32, name="x2ps2", tag="x2ps2", bufs=1)
            nc.tensor.matmul(x2ps2[:], mid2T[:, :, 128], sb["Wi2R"][:],
                             start=True, stop=True)
            nc.vector.tensor_copy(X2[0:64, 64, :], x2ps2[:])

        # ---------- stage 2 ----------
        with tc.tile_pool(name="s2sp", bufs=2) as sp, \
             tc.tile_pool(name="s2pp", space="PSUM", bufs=2) as pp:
            st = {"hT": zerobf, "cT": zerobf, "cH": zeros128[:, 0:64]}
            for t in range(T if 2 in only_stages else 0):
                enc_step(t, 2, sp, pp, st)
            for fn in st.pop("defer", []):
                fn()

        # ---------- stage 3 ----------
        with tc.tile_pool(name="s3sp", bufs=2) as sp, \
             tc.tile_pool(name="s3pp", space="PSUM", bufs=2) as pp:
            outps = pp.tile([64, 18], F32, name="outps", bufs=1) if 3 in only_stages else None
            st = {"hT": zerobf, "cT": zerobf, "cH": zeros128[:, 0:64]}
            for t in range(TD if 3 in only_stages else 0):
                hT_old, cT_old, cH_old = st["hT"], st["cT"], st["cH"]
                eps = pp.tile([128, H], F32, name="e3ps", tag="eps3", bufs=1)
                for gb in (0, 64):
                    o = eps[gb:gb + 64, :]
                    nc.tensor.matmul(o, ones1[:], sb["Wxb"][:], start=True, stop=False)
                    nc.tensor.matmul(o, cT_old[:], sb["WhRb"][:],
                                     start=False, stop=False)
                    nc.tensor.matmul(o, hT_old[:], sb["WhRa"][:],
                                     start=False, stop=True)
                esb = sp.tile([128, H], BF16, name="e3sb", tag="esb3", bufs=2)
                nc.vector.tensor_copy(esb[:], eps[:])

                gps = pp.tile([128, 4, 512], F32, name="g3ps", tag="g3ps", bufs=1)
                for q in range(4):
                    o = gps[:, q, 0:64]
                    nc.tensor.matmul(o, sb["bdrow"][:, 128 * q:128 * (q + 1)],
                                     ones1[:], start=True, stop=False)
                    nc.tensor.matmul(o, sb["GdH"][:, 128 * q:128 * (q + 1)],
                                     hT_old[:], start=False, stop=False)

                if skip_score:
                    score = sp.tile([128, 32], BF16, name="score", tag="d_score", bufs=2)
                    nc.vector.memset(score[:], 0.1)
                else:
                    score = score_chunked(WxF3, esb, sb["vdup3"], 32, H, sp, "d",
                                          nchunks=split3 if split3 is not None else split)
                ex, zr = softmax_nomax(score, sp, pp, 32, ptag="tps3", pbufs=2,
                                       want_a=False, exdt=BF16)

                # context from unnormalized ex (starts right after exp; the
                # fold/recip run in parallel), then scale uu by 1/Z
                # (per-partition) before the fold+transpose matmul.
                ym = sp.tile([128, H, 32], BF16, name="ym", tag="ym", bufs=1)
                nc.vector.tensor_mul(ym[:, 0:112, :], finB[:, 0:112, :],
                                     ex[:].unsqueeze(1).broadcast_to([128, 112, 32]))
                nc.gpsimd.tensor_mul(ym[:, 112:128, :], finB[:, 112:128, :],
                                     ex[:].unsqueeze(1).broadcast_to([128, 16, 32]))
                uu0 = sp.tile([128, H], BF16, name="uu0", tag="uu0", bufs=2)
                tree_to(uu0[:, 0:112], ym[:, 0:112, :], sp, "ctr", 112, 32, single_cut=0)
                tree_to(uu0[:, 112:128], ym[:, 112:128, :], sp, "ctrp", 16, 32,
                        single_cut=0, eng=nc.gpsimd)
                uu = sp.tile([128, H], BF16, name="uu", tag="uu", bufs=2)
                nc.vector.tensor_scalar_mul(uu[:], uu0[:], zr[:])
                dinps = pp.tile([128, 64], F32, name="dinps", tag="tps3", bufs=2)
                nc.tensor.matmul(dinps[:], uu[:], sb["I64bf"][:],
                                 start=True, stop=True)
                dinT = sp.tile([128, 64], BF16, name="dinT", tag="dinT", bufs=2)
                nc.vector.tensor_copy(dinT[:], dinps[:])

                # late gate part: din joins the open per-bank groups
                for q in range(4):
                    nc.tensor.matmul(gps[:, q, 0:64],
                                     sb["GdX"][:, 128 * q:128 * (q + 1)],
                                     dinT[:], start=False, stop=True,
                                     skip_group_check=True)

                h3 = sp.tile([128, 64], BF16, name="h3", tag="h3", bufs=2)
                cH, cTbf = lstm_hmajor(gps, cH_old, h3[:], sp, pfx="3")
                st["hT"], st["cT"], st["cH"] = h3, cTbf, cH

                if t >= TD - 18:
                    j = t - (TD - 18)
                    nc.tensor.matmul(outps[:, j:j + 1], h3[:], sb["regw"][:],
                                     start=True, stop=True)

            if 3 in only_stages:
                nc.vector.tensor_copy(outsb[:], outps[:])
            nc.sync.dma_start(out_d[:], outsb[:])

        wpool.release()

    nc.compile()
    return nc


_NC_CACHE = {}


def kernel(**inputs):
    if "nc" not in _NC_CACHE:
        _NC_CACHE["nc"] = build_nc()
    nc = _NC_CACHE["nc"]
    w = prep_weights({k: np.asarray(v) for k, v in inputs.items()})
    in_maps = []
    for core in range(N_CORES):
        m = dict(w)
        m.update(prep_core_inputs(inputs, core))
        in_maps.append(m)
    res = run_bass_kernel_spmd(nc, in_maps, list(range(N_CORES)))
    out = np.concatenate([res.results[c]["out"] for c in range(N_CORES)], axis=0)
    out = out + np.asarray(inputs["reg_b"])[0]
    return out.astype(np.float32)


# revision 34
# speedup vs baseline: 1.0129x; 1.0063x over previous
# DSTP-RNN Trainium2 kernel: 8-core pure data parallel (batch 512 -> 64/core).
#
# Layout summary:
#  - "Score" tensors are b-major: partitions = (g, b) with g a channel-group
#    split, b = 64 local batch rows; free dims = (ch, tau).
#  - LSTM runs H-major: gates land in PSUM [H=128, b=64] via per-quarter
#    matmuls (stationary = weight slices), so h/c are produced directly in
#    the [H, b] layout the next step's matmuls consume - no transposes on
#    the recurrent chain.  States are doubled (hS=2h, cS=2c) and sigmoids
#    are computed as tanh(x/2) with pre-scaled weights.
#  - All matmul operands are bf16 (1 cycle/row); elementwise stays f32.
#  - Gate accumulation is split: bias+h parts issue right after the e-matmul
#    (overlapping the attention score), x parts join at the end.
#  - Off-critical-path copies (finB, WxF, mid2T staging) go to ACT/Pool.
import numpy as np
import ml_dtypes

import concourse.bacc as bacc
import concourse.mybir as mybir
import concourse.tile as tile
from concourse.bass_utils import run_bass_kernel_spmd

F32 = mybir.dt.float32
BF16 = mybir.dt.bfloat16
AX = mybir.AxisListType
OP = mybir.AluOpType
AF = mybir.ActivationFunctionType

N_CORES = 8
B = 64      # batch per core
T = 64      # encoder length
H = 128
TD = 24     # decoder steps (T_DEC + 6)
NF = 17     # driving series count
C2 = 129    # stage-2 channels (H + label)
COLS = np.array(list(range(14)) + list(range(15, 18)))
PAD_NEG = -20.0   # pad channel fill (tanh -> -1; excluded from softmax sums)


def _perm_cols(w):
    # keep torch gate order (i,f,g,o): i,f,g contiguous so the chain-critical
    # tanh (i,f,g) can issue before the o quarter's matmuls finish
    return w


def _bf(x):
    return np.ascontiguousarray(np.asarray(x).astype(ml_dtypes.bfloat16))


def _f32(x):
    return np.ascontiguousarray(np.asarray(x).astype(np.float32))


def prep_weights(inp):
    w = {}
    w["Wi1R"] = _bf(np.concatenate([inp["Wi_w"].T, inp["Wi_b"][None, :]], 0))
    w["Wi2R"] = _bf(np.concatenate([inp["Wi2_w"].T * 0.5, inp["Wi2_b"][None, :]], 0))
    w["We1R"] = _bf(inp["We_w"].T * 0.5)
    w["We2R"] = _bf(inp["We2_w"].T * 0.5)
    w["WhR"] = _bf(inp["Wh_w"].T * 0.5)
    w["WxR"] = _bf(inp["Wx_w"].T * 0.5)
    w["Wxb"] = _bf(inp["Wx_b"][None, :])

    # ISO: sigmoid gates computed as tanh(x/2) -> pre-scale i,f,o cols by 0.5.
    # States are stored doubled (hS=2h, cS=2c), so weight blocks consuming
    # h/c/mid/din get an extra 0.5.
    ISO = np.concatenate([0.5 * np.ones(256), np.ones(128),
                          0.5 * np.ones(128)]).astype(np.float32)
    g1x = _perm_cols(inp["Wih1"].T) * ISO
    b1 = _perm_cols((inp["bih1"] + inp["bhh1"])[None, :]) * ISO
    w["G1XA"] = _bf(np.concatenate([g1x[0:9], b1], 0))   # +bias row (ones in x)
    w["G1XB"] = _bf(g1x[9:17])
    w["G1H"] = _bf(_perm_cols(inp["Whh1"].T) * ISO * 0.5)

    g2x = _perm_cols(inp["Wih2"].T) * ISO * 0.5
    b2 = _perm_cols((inp["bih2"] + inp["bhh2"])[None, :]) * ISO
    # stage-2 channel groups are chosen so x rows align with hT1 partitions:
    # group 0 = [h0..h63, label] (65 ch), group 1 = [h64..h127] (64 ch)
    w["G2XA"] = _bf(np.concatenate([g2x[0:64], g2x[128:129], b2], 0))  # +bias row
    w["G2XB"] = _bf(g2x[64:128])
    w["G2H"] = _bf(_perm_cols(inp["Whh2"].T) * ISO * 0.5)

    w["GdX"] = _bf(_perm_cols(inp["Wihd"].T) * ISO * 0.5)
    w["GdH"] = _bf(_perm_cols(inp["Whhd"].T) * ISO * 0.5)
    w["bdrow"] = _bf(_perm_cols((inp["bihd"] + inp["bhhd"])[None, :]) * ISO)

    w["vdup1"] = _bf(np.broadcast_to(inp["Vd_w"][0][None, :], (128, T)))
    w["vdup2"] = _bf(np.broadcast_to(inp["Vd2_w"][0][None, :], (128, T)))
    w["vdup3"] = _bf(np.broadcast_to(inp["V_w"][0][None, :], (128, H)))
    w["regw"] = _bf(inp["reg_w"][0][:, None] * 0.5)

    eye = np.eye(64, dtype=np.float32)
    w["I64dup"] = _f32(np.concatenate([eye, eye], 0))
    w["I64bf"] = _bf(np.concatenate([eye, eye], 0))
    w["I128bf"] = _bf(np.eye(128, dtype=np.float32))
    foldDup = (np.arange(128)[:, None] % 64 == np.arange(128)[None, :] % 64)
    w["foldDup"] = _f32(foldDup.astype(np.float32))
    w["onesrow"] = _bf(np.ones((1, 64), np.float32))
    return w


def prep_core_inputs(inp, core):
    b0, b1 = core * B, (core + 1) * B
    x = np.asarray(inp["input_p_q"])[b0:b1, :T, :][:, :, COLS]   # [64,64,17]
    lab = np.asarray(inp["label_p"])[b0:b1, :T]                  # [64,64]
    d = {}
    inpT = np.ones((65, NF * B), np.float32)
    inpT[:64] = x.transpose(1, 2, 0).reshape(64, NF * B)         # [t, (c,b)]
    d["inpT"] = _bf(inpT)
    ct = x.transpose(2, 1, 0).reshape(NF, T * B)                 # [c, (t,b)]
    d["inpCTA"] = _bf(ct[0:9])
    d["inpCTB"] = _bf(ct[9:17])
    d["labelT"] = _f32(lab.T * 2.0)                                    # [t, b]
    return d


DRAM_SPECS = {
    "inpT": ([65, NF * B], BF16), "inpCTA": ([9, T * B], BF16),
    "inpCTB": ([8, T * B], BF16), "labelT": ([T, B], F32),
    "Wi1R": ([65, 64], BF16), "Wi2R": ([65, 64], BF16),
    "We1R": ([256, 64], BF16), "We2R": ([256, 64], BF16),
    "WhR": ([256, 128], BF16), "WxR": ([128, 128], BF16), "Wxb": ([1, 128], BF16),
    "G1XA": ([10, 512], BF16), "G1XB": ([8, 512], BF16), "G1H": ([128, 512], BF16),
    "G2XA": ([66, 512], BF16), "G2XB": ([64, 512], BF16), "G2H": ([128, 512], BF16),
    "GdX": ([128, 512], BF16), "GdH": ([128, 512], BF16), "bdrow": ([1, 512], BF16),
    "vdup1": ([128, T], BF16), "vdup2": ([128, T], BF16), "vdup3": ([128, H], BF16),
    "regw": ([128, 1], BF16), "I64dup": ([128, 64], F32), "I64bf": ([128, 64], BF16),
    "I128bf": ([128, 128], BF16), "foldDup": ([128, 128], F32),
    "onesrow": ([1, 64], BF16),
}


def build_nc(num_devices=N_CORES, skip_score=False, skip_tail=False, only_stages=(1, 2, 3), split=(0.28, 0.64), split3=(0.25, 0.62)):
    nc = bacc.Bacc("TRN2", target_bir_lowering=False, debug=False,
                   num_devices=num_devices)
    dr = {}
    for name, (shape, dt) in DRAM_SPECS.items():
        dr[name] = nc.dram_tensor(name, shape, dt, kind="ExternalInput").ap()
    out_d = nc.dram_tensor("out", [B, 18], F32, kind="ExternalOutput").ap()

    with tile.TileContext(nc) as tc:
        # ---------- persistent SBUF ----------
        wpool = tc.alloc_tile_pool(name="wpool", bufs=1)
        sb = {}
        for name, (shape, dt) in DRAM_SPECS.items():
            if shape[0] > 128:
                assert shape[0] == 256
                for half, suf in ((0, "a"), (1, "b")):
                    key = name + suf
                    sb[key] = wpool.tile([128, shape[1]], dt, name=f"sb_{key}")
                    nc.sync.dma_start(sb[key][:], dr[name][128 * half:128 * (half + 1), :])
            else:
                sb[name] = wpool.tile(shape, dt, name=f"sb_{name}")
                nc.sync.dma_start(sb[name][:], dr[name][:])

        G2XBsh = wpool.tile([128, 512], BF16, name="G2XBsh")
        nc.sync.dma_start(G2XBsh[64:128, :], dr["G2XB"][:])
        X1 = wpool.tile([128, 9, T], BF16, name="X1")
        X2 = wpool.tile([128, 65, T], BF16, name="X2")
        WxF3 = wpool.tile([128, 32, H], BF16, name="WxF3")
        finB = wpool.tile([128, H, 32], BF16, name="finB")
        finT = wpool.tile([128, T, B], BF16, name="finT")    # stage-2 h, H-major
        hT1 = wpool.tile([128, T, B], BF16, name="hT1")      # stage-1 h, H-major
        labB = wpool.tile([65, T, B], BF16, name="labB")     # label at partition 64
        mid2T = wpool.tile([65, B, C2], BF16, name="mid2T")
        xA1pp = [wpool.tile([10, 64], BF16, name=f"xA1_{k}") for k in range(2)]
        xA2pp = [wpool.tile([66, 64], BF16, name=f"xA2_{k}") for k in range(2)]
        zeros128 = wpool.tile([128, 128], F32, name="zeros128")
        zerobf = wpool.tile([128, 64], BF16, name="zerobf")
        ones1 = wpool.tile([1, 64], BF16, name="ones1")
        outsb = wpool.tile([B, 18], F32, name="outsb")

        for k in range(2):
            nc.gpsimd.dma_start(xA1pp[k][9:10, :], dr["onesrow"][:])
            nc.gpsimd.dma_start(xA2pp[k][65:66, :], dr["onesrow"][:])
        nc.vector.memset(zerobf[:], 0.0)
        nc.vector.memset(zeros128[:], 0.0)
        nc.vector.memset(ones1[:], 1.0)
        nc.vector.memset(mid2T[64:65, :, :], 1.0)
        nc.vector.memset(X2[64:128, 64, :], PAD_NEG)
        nc.vector.memset(X1[64:128, 8, :], PAD_NEG)
        # label -> mid2T[t, b, 128] and labB[64, t, b]
        nc.gpsimd.dma_start(mid2T[0:64, :, 128:129], dr["labelT"][:])
        nc.gpsimd.dma_start(labB[64:65, :, :], dr["labelT"][:])

        if only_stages != (1, 2, 3):
            # profiling variants: init tiles a skipped stage would have written
            nc.vector.memset(finT[:], 0.1)
            nc.vector.memset(finB[:], 0.1)
            nc.vector.memset(hT1[:], 0.1)
            nc.vector.memset(mid2T[:], 0.1)
            nc.vector.memset(X2[:], 0.1)
            nc.vector.memset(X1[:], 0.1)
            nc.vector.memset(WxF3[:], 0.1)
            nc.vector.memset(outsb[:], 0.0)

        # ---------- X1 build ----------
        with tc.tile_pool(name="xb1", space="PSUM", bufs=1) as xb:
            x1ps = xb.tile([128, 9, T], F32, name="x1ps")
            for c in range(NF):
                g, ch = (0, c) if c < 9 else (1, c - 9)
                rows = slice(g * 64, g * 64 + 64)
                nc.tensor.matmul(x1ps[rows, ch, :],
                                 sb["inpT"][:, c * B:(c + 1) * B],
                                 sb["Wi1R"][:], start=True, stop=True)
            nc.vector.tensor_copy(X1[0:64, :, :], x1ps[0:64, :, :])
            nc.scalar.copy(X1[64:128, 0:8, :], x1ps[64:128, 0:8, :])

        # ================= helpers =================
        def lstm_hmajor(gps, cH_old, hdst, sp, pfx=""):
            """H-major doubled-state LSTM.  gps psum [128, 256] = [i|f|o|g]
            quarters, each [H=128, b=64].  Writes hS (bf16) into hdst and
            returns (cH_new f32, cTbf bf16)."""
            ta = sp.tile([128, 4, 64], F32, name=pfx + "ta", tag=pfx + "ta", bufs=2)
            # i,f,g first (feeds the c update); o separately (only needed by h)
            nc.scalar.activation(ta[:, 0:3, :], gps[:, 0:3, 0:64], AF.Tanh)
            nc.scalar.activation(ta[:, 3:4, :], gps[:, 3:4, 0:64], AF.Tanh)
            u = sp.tile([128, 64], F32, name=pfx + "u", tag=pfx + "u", bufs=2)
            v2 = sp.tile([128, 64], F32, name=pfx + "v2", tag=pfx + "v2", bufs=2)
            # u = (tanh(i/2)+1)*tanh(g) = 2*sig(i)*tanh(g)
            nc.vector.scalar_tensor_tensor(u[:], ta[:, 0, :], 1.0,
                                           ta[:, 2, :], op0=OP.add, op1=OP.mult)
            # v = (tanh(f/2)+1)*cS = 4*sig(f)*c
            nc.vector.scalar_tensor_tensor(v2[:], ta[:, 1, :], 1.0,
                                           cH_old[:], op0=OP.add, op1=OP.mult)
            cH = sp.tile([128, 64], F32, name=pfx + "cH", tag=pfx + "cH", bufs=2)
            # cS_new = v/2 + u = 2*c_new
            nc.vector.scalar_tensor_tensor(cH[:], v2[:], 0.5,
                                           u[:], op0=OP.mult, op1=OP.add)
            cTbf = sp.tile([128, 64], BF16, name=pfx + "cTb", tag=pfx + "cTb", bufs=2)
            nc.vector.tensor_copy(cTbf[:], cH[:])
            tcel = sp.tile([128, 64], F32, name=pfx + "tc", tag=pfx + "tc", bufs=2)
            nc.scalar.activation(tcel[:], cH[:], AF.Tanh, scale=0.5)
            # hS_new = (tanh(o/2)+1)*tanh(c) = 2*h_new
            nc.vector.scalar_tensor_tensor(hdst, ta[:, 3, :], 1.0,
                                           tcel[:], op0=OP.add, op1=OP.mult)
            return cH, cTbf

        def softmax_nomax(score, pool, ppool, nch, ptag="tps", pbufs=3,
                          want_a=True, exdt=F32):
            # score pad slots (if any) must already be ~-30 so exp ~ 0;
            # accum_out fuses the per-partition sum into the exp pass.
            ex = pool.tile([128, nch], exdt, name="ex", tag="sm_ex", bufs=2)
            zs = pool.tile([128, 1], F32, name="zs", tag="sm_zs", bufs=2)
            nc.scalar.activation(ex[:], score[:], AF.Exp, accum_out=zs[:])
            zps = ppool.tile([128, 1], F32, name="zps", tag=ptag, bufs=pbufs)
            nc.tensor.matmul(zps[:], sb["foldDup"][:], zs[:], start=True, stop=True)
            zr = pool.tile([128, 1], F32, name="zr", tag="sm_zr", bufs=2)
            nc.vector.reciprocal(zr[:], zps[:])
            if not want_a:
                return ex, zr
            a = pool.tile([128, nch], BF16, name="a", tag="sm_a", bufs=2)
            nc.vector.tensor_scalar_mul(a[:], ex[:], zr[:])
            return a

        def tree_to(dst, src, pool, tag, nch, ntau, single_cut=0, eng=None):
            """sum src [128, nch, ntau] over tau into dst [128, nch] slice."""
            if eng is None:
                eng = nc.vector
            if eng is not nc.vector:
                single_cut = 0   # Pool tensor_reduce can't do innermost-axis
            cur, n, lvl = src, ntau, 0
            while n > max(2, single_cut):
                n //= 2
                nxt = pool.tile([128, nch, n], BF16, name=f"{tag}_{lvl}",
                                tag=f"{tag}_{lvl}", bufs=1)
                eng.tensor_add(nxt[:], cur[:, :, 0:n], cur[:, :, n:2 * n])
                cur, lvl = nxt, lvl + 1
            if n > 2:
                with nc.allow_low_precision(reason="tiny bf16 tau-reduce"):
                    nc.vector.tensor_reduce(dst, cur[:], AX.X, OP.add)
            else:
                eng.tensor_add(dst.unsqueeze(-1), cur[:, :, 0:1], cur[:, :, 1:2])

        def score_chunked(Xs, esb, vdup, nch, ntau, sp, tag, pad_neg=False,
                          nchunks=2):
            """returns score [128, nch] bf16; chunks over ch for engine overlap."""
            score = sp.tile([128, nch], BF16, name="score", tag=f"{tag}_score",
                            bufs=2)
            if nchunks == 1:
                bounds = ((0, nch),)
            elif isinstance(nchunks, tuple):
                cuts = [0] + [max(1, min(nch - 1, int(round(nch * f)))) for f in nchunks] + [nch]
                bounds = tuple((cuts[i], cuts[i + 1]) for i in range(len(cuts) - 1))
            elif isinstance(nchunks, float):
                cut = max(1, min(nch - 1, int(round(nch * nchunks))))
                bounds = ((0, cut), (cut, nch))
            elif nchunks == 2:
                half = (nch + 1) // 2
                bounds = ((0, half), (half, nch))
            else:
                q = max(1, nch // nchunks)
                cuts = list(range(0, nch, q))
                bounds = tuple((lo, min(lo + q, nch)) for lo in cuts)
            for ci, (lo, hi) in enumerate(bounds):
                w = hi - lo
                scA = sp.tile([128, w, ntau], BF16, name="scA",
                              tag=f"{tag}_scA{lo}", bufs=1)
                nc.vector.tensor_add(scA[:], Xs[:, lo:hi, :],
                                     esb[:].unsqueeze(1).broadcast_to([128, w, ntau]))
                scT = sp.tile([128, w, ntau], BF16, name="scT",
                              tag=f"{tag}_scT{lo}", bufs=1)
                nc.scalar.activation(scT[:], scA[:], AF.Tanh)
                scM = sp.tile([128, w, ntau], BF16, name="scM",
                              tag=f"{tag}_scM{lo}", bufs=1)
                # the first chunk's result isn't needed until the exp, so its
                # mul+tree can run on the otherwise-idle Pool engine
                eng = nc.gpsimd if (ci == 0 and len(bounds) > 1) else nc.vector
                eng.tensor_mul(scM[:], scT[:],
                               vdup[:].unsqueeze(1).broadcast_to([128, w, ntau]))
                tree_to(score[:, lo:hi], scM, sp, f"{tag}_tr{lo}", w, ntau,
                        single_cut=16, eng=eng)
            if pad_neg:
                # kill the (g=1, ch=nch-1) pad slot before exp
                nc.vector.memset(score[64:128, nch - 1:nch], -30.0)
            return score

        # ================= encoder step =================
        def enc_step(t, stage, sp, pp, st):
            if stage == 1:
                Xs, vdup, WeRa, WeRb = X1, sb["vdup1"], sb["We1Ra"], sb["We1Rb"]
                nch = 9
                GH, GXA, GXB = sb["G1H"], sb["G1XA"], sb["G1XB"]
            else:
                Xs, vdup, WeRa, WeRb = X2, sb["vdup2"], sb["We2Ra"], sb["We2Rb"]
                nch = 65
                GH, GXA, GXB = sb["G2H"], sb["G2XA"], G2XBsh
            hT_old, cT_old, cH_old = st["hT"], st["cT"], st["cH"]
            tpsb = 2

            # e = [h;c] @ We  (b-major psum); c-part first (it's ready earlier)
            eps = pp.tile([128, T], F32, name="eps", tag="eps", bufs=1)
            for gb in (0, 64):
                o = eps[gb:gb + 64, :]
                nc.tensor.matmul(o, cT_old[:], WeRb[:], start=True, stop=False)
                nc.tensor.matmul(o, hT_old[:], WeRa[:], start=False, stop=True)
            esb = sp.tile([128, T], BF16, name="esb", tag="esb", bufs=2)
            nc.vector.tensor_copy(esb[:], eps[:])
            for fn in st.pop("defer", []):
                fn()

            # one bank per gate quarter: a start=True only zeroes its own bank
            gps = pp.tile([128, 4, 512], F32, name="gps", tag="gps", bufs=1)
            for q in range(4):
                nc.tensor.matmul(gps[:, q, 0:64], GH[:, 128 * q:128 * (q + 1)],
                                 hT_old[:], start=True, stop=False)

            if skip_score:
                score = sp.tile([128, nch], BF16, name="score", tag="e_score", bufs=2)
                nc.vector.memset(score[:], 0.1)
            else:
                score = score_chunked(Xs, esb, vdup, nch, T, sp, "e", pad_neg=True,
                                      nchunks=1 if stage == 1 else split)
            a = softmax_nomax(score, sp, pp, nch, pbufs=tpsb, exdt=BF16)

            if stage == 1:
                aTB = pp.tile([8, 64], BF16, name="aTB", tag="tps", bufs=tpsb)
                nc.tensor.transpose(aTB[:], a[64:128, 0:8], sb["I64bf"][64:128, :])
                aTA = pp.tile([9, 64], BF16, name="aTA", tag="tps", bufs=tpsb)
                nc.tensor.transpose(aTA[:], a[0:64, 0:9], sb["I64bf"][0:64, :])
                xB = sp.tile([8, 64], BF16, name="x1B", tag="xB", bufs=2)
                nc.vector.tensor_mul(xB[:], sb["inpCTB"][:, t * B:(t + 1) * B], aTB[:])
                xA = xA1pp[t % 2]
                nc.vector.tensor_mul(xA[0:9, :],
                                     sb["inpCTA"][:, t * B:(t + 1) * B], aTA[:])
            else:
                # group 0 = [h0..h63, label] at partitions 0..64,
                # group 1 = [h64..h127] at partitions 64..127 (psum base 64)
                aTB = pp.tile([64, 64], BF16, name="aTB", tag="tps", bufs=tpsb)
                nc.tensor.transpose(aTB[:], a[64:128, 0:64], sb["I64bf"][64:128, :])
                aTA = pp.tile([65, 64], BF16, name="aTA", tag="tps", bufs=tpsb)
                nc.tensor.transpose(aTA[:], a[0:64, 0:65], sb["I64bf"][0:64, :])
                xB = sp.tile([128, 64], BF16, name="x2B", tag="xB", bufs=2)
                nc.vector.tensor_mul(xB[64:128, :], hT1[64:128, t, :], aTB[0:64, :])
                xA = xA2pp[t % 2]
                nc.vector.tensor_mul(xA[0:64, :], hT1[0:64, t, :], aTA[0:64, :])
                nc.vector.tensor_mul(xA[64:65, :], labB[64:65, t, :], aTA[64:65, :])

            # late gate parts join the open per-bank groups; GXB first (xB is
            # the first x-mul to finish, so PE overlaps the remaining muls)
            for q in range(4):
                o = gps[:, q, 0:64]
                if stage == 1:
                    nc.tensor.matmul(o, GXB[:, 128 * q:128 * (q + 1)], xB[:],
                                     start=False, stop=False, skip_group_check=True)
                else:
                    nc.tensor.matmul(o, GXB[64:128, 128 * q:128 * (q + 1)],
                                     xB[64:128, :], start=False, stop=False,
                                     skip_group_check=True)
            for q in range(4):
                nc.tensor.matmul(gps[:, q, 0:64], GXA[:, 128 * q:128 * (q + 1)],
                                 xA[:], start=False, stop=True,
                                 skip_group_check=True)

            hdst = hT1[:, t, :] if stage == 1 else finT[:, t, :]
            cH, cTbf = lstm_hmajor(gps, cH_old, hdst, sp)
            st["hT"], st["cT"], st["cH"] = hdst, cTbf, cH

            # ---- off-critical-path per-step outputs; the ACT copies are
            # deferred past the next step's esb so they don't delay it ----
            if stage == 1:
                # mid2T[t] = h^T (b-major) for the X2 build
                def _defer(t=t, hdst=hdst):
                    hBps = pp.tile([64, 128], BF16, name="hBps", tag="hBps", bufs=1)
                    nc.tensor.transpose(hBps[:], hdst, sb["I128bf"][:])
                    hbf = sp.tile([64, 128], BF16, name="hbf", tag="hbf", bufs=2)
                    nc.scalar.copy(hbf[:], hBps[:])
                    if t % 2 == 0:
                        nc.sync.dma_start(mid2T[t:t + 1, :, 0:128], hbf[:])
                    else:
                        nc.gpsimd.dma_start(mid2T[t:t + 1, :, 0:128], hbf[:])
                st["defer"] = [_defer]
            else:
                g, sl = divmod(t, 32)
                rows = slice(g * 64, g * 64 + 64)
                # finB[(g,b), H, sl] = h^T  (transpose lands at base 0; the
                # copy shifts it to the group's partition block)
                def _defer(rows=rows, sl=sl, hdst=hdst):
                    fps = pp.tile([64, 128], BF16, name="fps", tag="fw", bufs=1)
                    nc.tensor.transpose(fps[:], hdst, sb["I128bf"][:])
                    wxps = pp.tile([128, 128], F32, name="wxps", tag="fw", bufs=1)
                    nc.tensor.matmul(wxps[rows, :], hdst, sb["WxR"][:],
                                     start=True, stop=True)
                    nc.scalar.copy(finB[rows, :, sl].unsqueeze(-1),
                                   fps[:].unsqueeze(-1))
                    nc.scalar.copy(WxF3[rows, sl, :], wxps[rows, :])
                st["defer"] = [_defer]

        # ---------- stage 1 ----------
        with tc.tile_pool(name="s1sp", bufs=2) as sp, \
             tc.tile_pool(name="s1pp", space="PSUM", bufs=2) as pp:
            st = {"hT": zerobf, "cT": zerobf, "cH": zeros128[:, 0:64]}
            for t in range(T if 1 in only_stages else 0):
                enc_step(t, 1, sp, pp, st)
            for fn in st.pop("defer", []):
                fn()

        # ---------- X2 build ----------
        with tc.tile_pool(name="xb2", space="PSUM", bufs=2) as xb2:
            for r in range(4):
                x2ps = xb2.tile([128, 16, T], F32, name="x2ps", tag="x2ps", bufs=2)
                for k in range(16):
                    ch = r * 16 + k
                    nc.tensor.matmul(x2ps[0:64, k, :], mid2T[:, :, ch],
                                     sb["Wi2R"][:], start=True, stop=True)
                    nc.tensor.matmul(x2ps[64:128, k, :], mid2T[:, :, 64 + ch],
                                     sb["Wi2R"][:], start=True, stop=True)
                nc.vector.tensor_copy(X2[:, r * 16:(r + 1) * 16, :], x2ps[:])
            x2ps2 = xb2.tile([64, T], F32, name="x2ps2", tag="x2ps2", bufs=1)
            nc.tensor.matmul(x2ps2[:], mid2T[:, :, 128], sb["Wi2R"][:],
                             start=True, stop=True)
            nc.vector.tensor_copy(X2[0:64, 64, :], x2ps2[:])

        # ---------- stage 2 ----------
        with tc.tile_pool(name="s2sp", bufs=2) as sp, \
             tc.tile_pool(name="s2pp", space="PSUM", bufs=2) as pp:
            st = {"hT": zerobf, "cT": zerobf, "cH": zeros128[:, 0:64]}
            for t in range(T if 2 in only_stages else 0):
                enc_step(t, 2, sp, pp, st)
            for fn in st.pop("defer", []):
                fn()

        # ---------- stage 3 ----------
        with tc.tile_pool(name="s3sp", bufs=2) as sp, \
             tc.tile_pool(name="s3pp", space="PSUM", bufs=2) as pp:
            outps = pp.tile([64, 18], F32, name="outps", bufs=1) if 3 in only_stages else None
            st = {"hT": zerobf, "cT": zerobf, "cH": zeros128[:, 0:64]}
            for t in range(TD if 3 in only_stages else 0):
                hT_old, cT_old, cH_old = st["hT"], st["cT"], st["cH"]
                eps = pp.tile([128, H], F32, name="e3ps", tag="eps3", bufs=1)
                for gb in (0, 64):
                    o = eps[gb:gb + 64, :]
                    nc.tensor.matmul(o, ones1[:], sb["Wxb"][:], start=True, stop=False)
                    nc.tensor.matmul(o, cT_old[:], sb["WhRb"][:],
                                     start=False, stop=False)
                    nc.tensor.matmul(o, hT_old[:], sb["WhRa"][:],
                                     start=False, stop=True)
                esb = sp.tile([128, H], BF16, name="e3sb", tag="esb3", bufs=2)
                nc.vector.tensor_copy(esb[:], eps[:])

                gps = pp.tile([128, 4, 512], F32, name="g3ps", tag="g3ps", bufs=1)
                for q in range(4):
                    o = gps[:, q, 0:64]
                    nc.tensor.matmul(o, sb["bdrow"][:, 128 * q:128 * (q + 1)],
                                     ones1[:], start=True, stop=False)
                    nc.tensor.matmul(o, sb["GdH"][:, 128 * q:128 * (q + 1)],
                                     hT_old[:], start=False, stop=False)

                if skip_score:
                    score = sp.tile([128, 32], BF16, name="score", tag="d_score", bufs=2)
                    nc.vector.memset(score[:], 0.1)
                else:
                    score = score_chunked(WxF3, esb, sb["vdup3"], 32, H, sp, "d",
                                          nchunks=split3 if split3 is not None else split)
                ex, zr = softmax_nomax(score, sp, pp, 32, ptag="tps3", pbufs=2,
                                       want_a=False, exdt=BF16)

                # context from unnormalized ex (starts right after exp; the
                # fold/recip run in parallel), then scale uu by 1/Z
                # (per-partition) before the fold+transpose matmul.
                ym = sp.tile([128, H, 32], BF16, name="ym", tag="ym", bufs=1)
                nc.vector.tensor_mul(ym[:, 0:112, :], finB[:, 0:112, :],
                                     ex[:].unsqueeze(1).broadcast_to([128, 112, 32]))
                nc.gpsimd.tensor_mul(ym[:, 112:128, :], finB[:, 112:128, :],
                                     ex[:].unsqueeze(1).broadcast_to([128, 16, 32]))
                uu0 = sp.tile([128, H], BF16, name="uu0", tag="uu0", bufs=2)
                tree_to(uu0[:, 0:112], ym[:, 0:112, :], sp, "ctr", 112, 32, single_cut=0)
                tree_to(uu0[:, 112:128], ym[:, 112:128, :], sp, "ctrp", 16, 32,
                        single_cut=0, eng=nc.gpsimd)
                uu = sp.tile([128, H], BF16, name="uu", tag="uu", bufs=2)
                nc.vector.tensor_scalar_mul(uu[:], uu0[:], zr[:])
                dinps = pp.tile([128, 64], F32, name="dinps", tag="tps3", bufs=2)
                nc.tensor.matmul(dinps[:], uu[:], sb["I64bf"][:],
                                 start=True, stop=True)
                dinT = sp.tile([128, 64], BF16, name="dinT", tag="dinT", bufs=2)
                nc.vector.tensor_copy(dinT[:], dinps[:])

                # late gate part: din joins the open per-bank groups
                for q in range(4):
                    nc.tensor.matmul(gps[:, q, 0:64],
                                     sb["GdX"][:, 128 * q:128 * (q + 1)],
                                     dinT[:], start=False, stop=True,
                                     skip_group_check=True)

                h3 = sp.tile([128, 64], BF16, name="h3", tag="h3", bufs=2)
                cH, cTbf = lstm_hmajor(gps, cH_old, h3[:], sp, pfx="3")
                st["hT"], st["cT"], st["cH"] = h3, cTbf, cH

                if t >= TD - 18:
                    j = t - (TD - 18)
                    nc.tensor.matmul(outps[:, j:j + 1], h3[:], sb["regw"][:],
                                     start=True, stop=True)

            if 3 in only_stages:
                nc.vector.tensor_copy(outsb[:], outps[:])
            nc.sync.dma_start(out_d[:], outsb[:])

        wpool.release()

    nc.compile()
    return nc


_NC_CACHE = {}


def kernel(**inputs):
    if "nc" not in _NC_CACHE:
        _NC_CACHE["nc"] = build_nc()
    nc = _NC_CACHE["nc"]
    w = prep_weights({k: np.asarray(v) for k, v in inputs.items()})
    in_maps = []
    for core in range(N_CORES):
        m = dict(w)
        m.update(prep_core_inputs(inputs, core))
        in_maps.append(m)
    res = run_bass_kernel_spmd(nc, in_maps, list(range(N_CORES)))
    out = np.concatenate([res.results[c]["out"] for c in range(N_CORES)], axis=0)
    out = out + np.asarray(inputs["reg_b"])[0]
    return out.astype(np.float32)
